# revision 48
# baseline (speedup 1.0000x reference)
"""Trainium2 Bass kernel for nn_Attention_32650341384246.

Full attention layer: qkv proj + per-head RMSNorm(q,k) + RoPE + softmax
attention (non-causal) + out proj.  B=2, S=2048, D=1024, H=16, DH=64.

Sharding: 8 cores; core c handles batch c//4, heads [4*(c%4), 4*(c%4)+4)
(data parallel over batch x tensor parallel over heads).  Each core
computes a partial [S, D] output (its heads @ Wout row-slice); the host
sums the 4 partials per batch and adds the (folded) biases.

Device design (per core):
  - x fed pre-transposed+bf16 as xT [128, 8, 2048]  (p + 128*a = model dim)
  - qkv proj emits qT/kT head-major [128 (2 heads x 64), S] directly
    (lhsT = W slice, rhs = xT slice) and v s-major [s, 4*64].
  - RMSNorm in head-major layout: sum(x^2) over d via ones-block matmul
    (f32r), rsqrt = Exp(-0.5*Ln(mean+eps)) on ACT (same table set as the
    softmax Exp -> zero table switches), partition-broadcast via ones
    matmul.
  - RoPE as q_rot = cosT*u + sinT'*swap(u); swap = adjacent-partition
    permutation matmul; cos/sin tables host-built from `pos` with
    q_scale/k_scale folded in; 1/sqrt(dh) folded into the exp scale.
  - scores^T [k, q] bf16 matmuls (K=64, tile_position row groups),
    PSUM [128, 1024] per head, staggered h0/h1 so ACT exp pipelines
    against PE; exp reads PSUM, writes bf16.
  - AV via lhsT = [v | ones] bf16 (M=65): row 64 accumulates sumexp.
  - normalize: gather 4 sumexp rows -> one DVE reciprocal [4, 512],
    select-matrix matmul broadcasts reciprocal rows across partitions.
  - out proj: lhsT = v_mixT bf16, rhs = Wout row-slice bf16.
Heavy matmuls are bf16 (fp32 PSUM accumulate); small helper matmuls
(sumsq / broadcasts / swap) stay float32r.
"""
import sys, os

sys.path.insert(0, "/opt/trn_rl_repo")

import numpy as np
from contextlib import ExitStack

import ml_dtypes
import concourse.bass as bass
import concourse.mybir as mybir
import concourse.tile as tile
from concourse import bacc
from concourse import bass_utils

F32 = mybir.dt.float32
F32R = mybir.dt.float32r
BF16 = mybir.dt.bfloat16
I16 = mybir.dt.int16
AF = mybir.ActivationFunctionType
ALU = mybir.AluOpType

# Schraudolph exp in bf16 bits: bf16(y) ~= exp(u) when int16(y) = A*u + B.
# A = 2^7/ln2; B centered to split the sawtooth error symmetrically.
SCHRAUD_A = 184.6650279
SCHRAUD_B = 16251.0

B, S, DM, H, DH = 2, 2048, 1024, 16, 64
NC = 8
HPC = H // 4          # 4 heads per core
HD = HPC * DH         # 256
NDT = DM // 128       # 8 model-dim tiles
THETA, EPS = 10000.0, 1e-6

LAST_RESULTS = None   # BassKernelResults of the most recent device run
_CACHED = {}


def build_program(exp_scale: float, shared_tables: bool):
    # Schraudolph offload is only range-proven for the rms-normed shared-scale
    # path (|s/sqrt(dh)| <= 8 keeps the int16 bits in [14.7k, 17.8k]).
    use_dve_exp = shared_tables
    nc = bacc.Bacc("TRN2", target_bir_lowering=False, debug=False)

    xT_d = nc.dram_tensor("xT", [128, NDT, S], BF16, kind="ExternalInput")
    w_d = nc.dram_tensor("w_all", [128, NDT, 3 * HD], BF16, kind="ExternalInput")
    wout_d = nc.dram_tensor("wout", [128, 2, DM], BF16, kind="ExternalInput")
    bq_d = nc.dram_tensor("bq", [128, 2], F32, kind="ExternalInput")
    bk_d = nc.dram_tensor("bk", [128, 2], F32, kind="ExternalInput")
    cosk_d = nc.dram_tensor("cos_k", [128, S], F32, kind="ExternalInput")
    sink_d = nc.dram_tensor("sin_k", [128, S], F32, kind="ExternalInput")
    if not shared_tables:
        cosq_d = nc.dram_tensor("cos_q", [128, S], F32, kind="ExternalInput")
        sinq_d = nc.dram_tensor("sin_q", [128, S], F32, kind="ExternalInput")
    P_d = nc.dram_tensor("Pswap", [128, 128], BF16, kind="ExternalInput")
    ob_d = nc.dram_tensor("onesblk", [128, 128], BF16, kind="ExternalInput")
    o2_d = nc.dram_tensor("ones2blk", [128, 128], BF16, kind="ExternalInput")
    sel_d = nc.dram_tensor("sel", [128, 2, 128], F32R, kind="ExternalInput")
    out_d = nc.dram_tensor("outp", [S, DM], BF16, kind="ExternalOutput")

    with tile.TileContext(nc) as tc, ExitStack() as ctx, \
            nc.allow_low_precision(reason="fp32r/bf16 matmul inputs"):
        singles = ctx.enter_context(tc.tile_pool(name="singles", bufs=1))
        tmp = ctx.enter_context(tc.tile_pool(name="tmp", bufs=2))
        expp = ctx.enter_context(tc.tile_pool(name="expp", bufs=4))
        outp = ctx.enter_context(tc.tile_pool(name="outp", bufs=4))

        # --- first-needed loads up front, finest-grained tiles so compute can
        # start as soon as the first s-chunk of x and the q/k weights land ---
        w_qk = [singles.tile([128, 2 * HD], BF16, name=f"wqk{dt}") for dt in range(NDT)]
        w_v = [singles.tile([128, HD], BF16, name=f"wv{dt}") for dt in range(NDT)]
        x_dt = [singles.tile([128, S], BF16, name=f"x{dt}") for dt in range(NDT)]
        for dt in range(NDT):
            nc.sync.dma_start(out=w_qk[dt], in_=w_d.ap()[:, dt, 0:2 * HD])
            nc.sync.dma_start(out=x_dt[dt], in_=xT_d.ap()[:, dt, :])
        for dt in range(NDT):
            nc.sync.dma_start(out=w_v[dt], in_=w_d.ap()[:, dt, 2 * HD:3 * HD])

        wout = singles.tile([128, 2, DM], BF16)
        nc.sync.dma_start(out=wout, in_=wout_d.ap())
        bq = singles.tile([128, 2], F32)
        nc.sync.dma_start(out=bq, in_=bq_d.ap())
        bk = singles.tile([128, 2], F32)
        nc.sync.dma_start(out=bk, in_=bk_d.ap())
        cos_k = singles.tile([128, S], F32)
        nc.sync.dma_start(out=cos_k, in_=cosk_d.ap())
        sin_k = singles.tile([128, S], F32)
        nc.sync.dma_start(out=sin_k, in_=sink_d.ap())
        if shared_tables:
            cos_q, sin_q = cos_k, sin_k
        else:
            cos_q = singles.tile([128, S], F32)
            nc.sync.dma_start(out=cos_q, in_=cosq_d.ap())
            sin_q = singles.tile([128, S], F32)
            nc.sync.dma_start(out=sin_q, in_=sinq_d.ap())
        Pm = singles.tile([128, 128], BF16)
        nc.sync.dma_start(out=Pm, in_=P_d.ap())
        onesblk = singles.tile([128, 128], BF16)
        nc.sync.dma_start(out=onesblk, in_=ob_d.ap())
        ones2blk = singles.tile([128, 128], BF16)
        nc.sync.dma_start(out=ones2blk, in_=o2_d.ap())
        sel = singles.tile([128, 2, 128], F32R)
        nc.sync.dma_start(out=sel, in_=sel_d.ap())
        eps_t = singles.tile([128, 1], F32)
        nc.vector.memset(eps_t, EPS)

        qt = [[singles.tile([128, 512], BF16, name=f"qt{t}_{sc}")
               for sc in range(4)] for t in range(2)]
        # k tiles zero-padded per head so scores run as full K=128 matmuls
        # (the other head's partitions hit zero weights) -> phase 2 never
        # switches PE tiling mode.
        ktz = [[[singles.tile([128, 512], BF16, name=f"ktz{t}_{sc}_{h}")
                 for h in range(2)] for sc in range(4)] for t in range(2)]
        for t in range(2):
            for sc in range(4):
                nc.gpsimd.memset(ktz[t][sc][0][64:128, :], 0.0)
                nc.gpsimd.memset(ktz[t][sc][1][0:64, :], 0.0)
        vhat = [singles.tile([128, 4, HPC, 65], BF16, name=f"vhat{sc}")
                for sc in range(4)]
        for sc in range(4):
            nc.vector.memset(vhat[sc][:, :, :, 64:65], 1.0)
        vmix = [[singles.tile([128, 1024], BF16, name=f"vmix{t}_{qh}")
                 for qh in range(2)] for t in range(2)]
        # rs (rsqrt) buffers: zeroed once; Exp writes rows 0-1, rows 2-127
        # stay 0 so the padded K=128 broadcast matmul reads finite zeros.
        rs_bufs = [singles.tile([128, 512], BF16, name=f"rs{i}")
                   for i in range(8)]
        for r in rs_bufs:
            nc.gpsimd.memset(r, 0.0)

        # ---------------- phase 1: qkv + rmsnorm + rope ----------------
        # rope factored as dest = (tt*cos + swap(tt)*sin) * rsqrt_broadcast:
        # the heavy rope work (swap matmul + 3 DVE ops) depends only on tt and
        # pipelines chunk-by-chunk inside stage A; only the final multiply
        # waits for the batched rsqrt.  Per group of 2 sections: stage A
        # (qkv, bias, square, sumsq, rope_raw; Ln lagged 2 chunks so the ACT
        # queue never head-blocks), stage B (8 batched Exps -> one Ln/Exp
        # table swap per group), stage C (rsqrt broadcast + final multiply).
        # Emission g0A g0B g1A g0C g1B v g1C keeps PE dense.  All matmuls are
        # full 128x128 mode (operands zero-padded) -> no PE mode switches.
        sections = (
            ("k", 0, bk, cos_k, sin_k),
            ("q", 0, bq, cos_q, sin_q),
            ("k", 1, bk, cos_k, sin_k),
            ("q", 1, bq, cos_q, sin_q))
        rraws, lnss = {}, {}
        ln_insts = {0: [], 1: []}
        exp_insts = {0: [], 1: []}
        with tc.tile_pool(name="ps1", bufs=1, space="PSUM") as ps1:
            def emit_ln(which, t, sc, gi):
                lns = tmp.tile([2, 512], BF16, tag="lns", bufs=8,
                               name=f"lns{which}{t}_{sc}")
                li = nc.scalar.activation(lns[:, :],
                                          lnss.pop((which, t, sc))[0:2, :],
                                          AF.Ln, bias=eps_t[0:2, :],
                                          scale=1.0 / DH)
                ln_insts[gi].append(li)
                lnss[(which, t, sc, "ln")] = lns

            def stage_a(group, gi):
                todo = []
                for which, t, bias, cosT, sinT in group:
                    off = 0 if which == "q" else HD
                    for sc in range(4):
                        if len(todo) >= 2:   # lag Ln 2 chunks: its pss is long
                            emit_ln(*todo.pop(0), gi)   # done -> no head-block
                        s0 = sc * 512
                        pq = ps1.tile([128, 512], F32, tag="acc", bufs=4,
                                      name=f"pq{which}{t}_{sc}")
                        for dt in range(NDT):
                            nc.tensor.matmul(
                                pq[:, :],
                                w_qk[dt][:, off + t * 128: off + (t + 1) * 128],
                                x_dt[dt][:, s0:s0 + 512],
                                start=(dt == 0), stop=(dt == NDT - 1))
                        tt = tmp.tile([128, 512], BF16, tag="tt", bufs=3,
                                      name=f"tt{which}{t}_{sc}")
                        nc.scalar.activation(tt[:, :], pq[:, :], AF.Identity,
                                             bias=bias[:, t:t + 1], scale=1.0)
                        sq = tmp.tile([128, 512], BF16, tag="sq", bufs=2,
                                      name=f"sq{which}{t}_{sc}")
                        nc.gpsimd.tensor_mul(sq[:, :], tt[:, :], tt[:, :])
                        pss = ps1.tile([128, 512], F32, tag="acc", bufs=4,
                                       name=f"pss{which}{t}_{sc}")
                        nc.tensor.matmul(pss[:, :], onesblk[:, :], sq[:, :],
                                         start=True, stop=True)
                        lnss[(which, t, sc)] = pss
                        psw = ps1.tile([128, 512], F32, tag="work", bufs=4,
                                       name=f"psw{which}{t}_{sc}")
                        nc.tensor.matmul(psw[:, :], Pm[:, :], tt[:, :],
                                         start=True, stop=True)
                        t1 = tmp.tile([128, 512], F32, tag="t1", bufs=2,
                                      name=f"t1{which}{t}_{sc}")
                        nc.vector.tensor_mul(t1[:, :], tt[:, :],
                                             cosT[:, s0:s0 + 512])
                        t2 = tmp.tile([128, 512], F32, tag="t2", bufs=2,
                                      name=f"t2{which}{t}_{sc}")
                        nc.vector.tensor_mul(t2[:, :], psw[:, :],
                                             sinT[:, s0:s0 + 512])
                        rr = tmp.tile([128, 512], BF16, tag="rr", bufs=16,
                                      name=f"rr{which}{t}_{sc}")
                        nc.vector.tensor_add(rr[:, :], t1[:, :], t2[:, :])
                        rraws[(which, t, sc)] = rr
                        todo.append((which, t, sc))
                for item in todo:
                    emit_ln(*item, gi)

            def stage_b(group, gi):
                for j, (which, t, _, _, _) in enumerate(group):
                    for sc in range(4):
                        rs = rs_bufs[j * 4 + sc]
                        ei = nc.scalar.activation(
                            rs[0:2, :], lnss.pop((which, t, sc, "ln"))[:, :],
                            AF.Exp, scale=-0.5)
                        exp_insts[gi].append(ei)

            def stage_c(group):
                for j, (which, t, _, _, _) in enumerate(group):
                    for sc in range(4):
                        rs = rs_bufs[j * 4 + sc]
                        pb = ps1.tile([128, 512], F32, tag="work", bufs=4,
                                      name=f"pb{which}{t}_{sc}")
                        nc.tensor.matmul(pb[:, :], ones2blk[:, :], rs[:, :],
                                         start=True, stop=True)
                        rr = rraws.pop((which, t, sc))
                        if which == "k":
                            nc.vector.tensor_mul(ktz[t][sc][0][0:64, :],
                                                 rr[0:64, :], pb[0:64, :])
                            nc.vector.tensor_mul(ktz[t][sc][1][64:128, :],
                                                 rr[64:128, :], pb[64:128, :])
                        else:
                            nc.vector.tensor_mul(qt[t][sc][:, :],
                                                 rr[:, :], pb[:, :])

            def v_section():
                for sc in range(4):
                    for st in range(4):
                        pv = ps1.tile([128, HD], F32, tag="work", bufs=4,
                                      name=f"pv{sc}_{st}")
                        for dt in range(NDT):
                            nc.tensor.matmul(
                                pv[:, :],
                                x_dt[dt][:, sc * 512 + st * 128: sc * 512 + (st + 1) * 128],
                                w_v[dt][:, :],
                                start=(dt == 0), stop=(dt == NDT - 1))
                        nc.vector.tensor_copy(
                            vhat[sc][:, st, :, 0:64],
                            pv[:, :].rearrange("p (h d) -> p h d", h=HPC))

            g0, g1 = sections[0:2], sections[2:4]
            stage_a(g0, 0)
            stage_b(g0, 0)
            stage_a(g1, 1)
            stage_c(g0)
            stage_b(g1, 1)
            v_section()
            stage_c(g1)

        # Tile's scheduler may interleave ACT instructions across batches,
        # ping-ponging the Ln/Exp table sets (they do not share a set on this
        # target).  Pin the order: g0 lns -> g0 exps -> g1 lns -> g1 exps.
        for ei in exp_insts[0]:
            tile.add_dep_helper(ei.ins, ln_insts[0][-1].ins, sync=False,
                                reason="g0 exps after g0 lns (ACT tables)")
        for li in ln_insts[1]:
            tile.add_dep_helper(li.ins, exp_insts[0][-1].ins, sync=False,
                                reason="g1 lns after g0 exps (ACT tables)")
        for ei in exp_insts[1]:
            tile.add_dep_helper(ei.ins, ln_insts[1][-1].ins, sync=False,
                                reason="g1 exps after g1 lns (ACT tables)")

        # ---------------- phase 2 + 3: attention, out proj per q-half ----------
        # exp split: h0 -> ACT exact exp, h1 -> DVE Schraudolph bit-trick exp
        # (bf16 bits = int16(A*u + B); safe since u = s/sqrt(dh) in [-8, 8]).
        # gpsimd (Pool) absorbs all PSUM->SBUF gather copies.  Out-proj for
        # q rows [qh*1024, +1024) runs right after both pairs finish that qh,
        # reusing the av PSUM banks, so its tail hides under qh1 attention.
        from concourse.dve_ops import (RECIP_APPROX_FAST_CONSTS,
                                       RECIPROCAL_APPROX_FAST)
        _c = RECIP_APPROX_FAST_CONSTS
        se = singles.tile([128, 512], F32, name="se_t")
        nc.gpsimd.memset(se, 1.0)
        with tc.tile_pool(name="ps2", bufs=1, space="PSUM") as ps2:
            for qh in range(2):
                for pair in range(2):
                    q0 = qh * 1024
                    ps_sc = [ps2.tile([128, 1024], F32, tag=f"sc{h}",
                                      name=f"sc{pair}{qh}{h}") for h in range(2)]
                    ps_av = [[ps2.tile([65, 512], F32, tag=f"av{h}{qc}",
                                       name=f"av{pair}{qh}{h}{qc}")
                              for qc in range(2)] for h in range(2)]
                    for kt in range(16):
                        for qc in range(2):
                            for h in range(2):
                                nc.tensor.matmul(
                                    ps_sc[h][:, qc * 512:(qc + 1) * 512],
                                    ktz[pair][kt // 4][h][:, (kt % 4) * 128:(kt % 4 + 1) * 128],
                                    qt[pair][qh * 2 + qc][:, :],
                                    start=True, stop=True)
                        es = []
                        for h in range(2):
                            e = expp.tile([128, 1024], BF16, tag=f"e{h}",
                                          name=f"e{pair}{qh}{h}_{kt}")
                            dve = h == 1 and kt % 3 == 2 and use_dve_exp
                            for qc in range(2):
                                cs = slice(qc * 512, (qc + 1) * 512)
                                if dve:
                                    nc.vector.tensor_scalar(
                                        e[:, cs].bitcast(I16), ps_sc[h][:, cs],
                                        SCHRAUD_A * exp_scale, SCHRAUD_B,
                                        op0=ALU.mult, op1=ALU.add)
                                else:
                                    xi = nc.scalar.activation(e[:, cs],
                                                              ps_sc[h][:, cs],
                                                              AF.Exp,
                                                              scale=exp_scale)
                                    tile.add_dep_helper(
                                        xi.ins, exp_insts[1][-1].ins, sync=False,
                                        reason="phase2 exps after g1 exps (ACT tables)")
                            es.append(e)
                        for h in range(2):
                            head = 2 * pair + h
                            for qc in range(2):
                                nc.tensor.matmul(
                                    ps_av[h][qc][:, :],
                                    vhat[kt // 4][:, kt % 4, head, :],
                                    es[h][:, qc * 512:(qc + 1) * 512],
                                    start=(kt == 0), stop=(kt == 15),
                                    skip_group_check=True)
                    # normalize: batch the 4 sumexp rows -> one reciprocal
                    # (rows live at 32-aligned partitions; rest stay 1.0
                    # so the reciprocal is finite and sel rows zero them)
                    for h in range(2):
                        for qc in range(2):
                            r0 = 32 * (2 * h + qc)
                            nc.vector.tensor_copy(se[r0:r0 + 1, :],
                                                  ps_av[h][qc][64:65, :])
                    recip4 = tmp.tile([128, 512], F32R, tag="recip4",
                                      name=f"rc{pair}{qh}")
                    nc.vector._custom_dve(RECIPROCAL_APPROX_FAST,
                                          out=recip4[:, :], in0=se[:, :],
                                          s0=_c["s0"], s1=_c["s1"],
                                          imm2=_c["imm2"])
                    for qc in range(2):
                        avs2 = tmp.tile([128, 512], F32, tag="avs2", bufs=2,
                                        name=f"avs{pair}{qh}{qc}")
                        for h in range(2):
                            nc.vector.tensor_copy(avs2[h * 64:(h + 1) * 64, :],
                                                  ps_av[h][qc][0:64, :])
                        pb2 = ps2.tile([128, 512], F32, tag=f"av0{qc}",
                                       name=f"nb{pair}{qh}{qc}")
                        nc.tensor.matmul(pb2[:, :], sel[:, qc, :], recip4[:, :],
                                         start=True, stop=True)
                        nc.vector.tensor_mul(
                            vmix[pair][qh][:, qc * 512:(qc + 1) * 512],
                            avs2[:, :], pb2[:, :])
                # out proj for this q-half; po rotates through the av banks
                for stl in range(8):
                    st = qh * 8 + stl
                    for n in range(2):
                        idx = stl * 2 + n
                        po = ps2.tile([128, 512], F32,
                                      tag=f"av{(idx % 4) // 2}{idx % 2}",
                                      name=f"po{st}_{n}")
                        for t in range(2):
                            nc.tensor.matmul(
                                po[:, :],
                                vmix[t][qh][:, stl * 128:(stl + 1) * 128],
                                wout[:, t, n * 512:(n + 1) * 512],
                                start=(t == 0), stop=(t == 1))
                        o = outp.tile([128, 512], BF16, tag="o", name=f"o{st}_{n}")
                        nc.vector.tensor_copy(o[:, :], po[:, :])
                        nc.sync.dma_start(
                            out=out_d.ap()[st * 128:(st + 1) * 128,
                                           n * 512:(n + 1) * 512],
                            in_=o[:, :])

    nc.compile()
    return nc


def host_prep(x, pos, Wqkv, bqkv, Wout, bout, q_scale, k_scale):
    """Build per-core input maps + shared-table decision."""
    x = np.asarray(x, dtype=np.float32)
    pos = np.asarray(pos, dtype=np.float32).reshape(-1)
    Wqkv = np.asarray(Wqkv, dtype=np.float32)
    bqkv = np.asarray(bqkv, dtype=np.float32)
    Wout = np.asarray(Wout, dtype=np.float32)
    q_scale = np.asarray(q_scale, dtype=np.float32)
    k_scale = np.asarray(k_scale, dtype=np.float32)

    shared = bool(np.array_equal(q_scale, k_scale))
    exp_scale = (1.0 / np.sqrt(DH)) if shared else 1.0

    # rope base tables [128, S]
    i_of_p = (np.arange(128) % 64) // 2            # pair index
    sign = np.where(np.arange(128) % 2 == 0, 1.0, -1.0)
    omega = THETA ** (-np.arange(0, DH, 2, dtype=np.float64) / DH)  # [32]
    ang = pos[None, :].astype(np.float64) * omega[:, None]          # [32, S]
    cosb = np.cos(ang)[i_of_p, :]                  # [128, S]
    sinb = np.sin(ang)[i_of_p, :] * sign[:, None]

    def tables(scale_vec, extra):
        sv = np.tile(scale_vec, 2)                 # [128]
        svx = np.tile(scale_vec[np.arange(64) ^ 1], 2)
        cosT = (cosb * sv[:, None] * extra).astype(np.float32)
        sinT = (sinb * svx[:, None] * extra).astype(np.float32)
        return np.ascontiguousarray(cosT), np.ascontiguousarray(sinT)

    cos_k, sin_k = tables(k_scale, 1.0)
    if not shared:
        cos_q, sin_q = tables(q_scale, 1.0 / np.sqrt(DH))

    Pm = np.zeros((128, 128), dtype=ml_dtypes.bfloat16)
    Pm[np.arange(128), np.arange(128) ^ 1] = 1.0
    # zero-padded to full 128x128 so the helper matmuls stay in 128x128 mode
    onesblk = np.zeros((128, 128), dtype=ml_dtypes.bfloat16)
    onesblk[0:64, 0] = 1.0
    onesblk[64:128, 1] = 1.0
    ones2blk = np.zeros((128, 128), dtype=ml_dtypes.bfloat16)
    ones2blk[0, 0:64] = 1.0
    ones2blk[1, 64:128] = 1.0
    # sel[qc]: [4, 128] selecting reciprocal row (h, qc) for partitions h*64..
    sel = np.zeros((128, 2, 128), dtype=np.float32)
    for qc in range(2):
        for h in range(2):
            sel[32 * (2 * h + qc), qc, h * 64:(h + 1) * 64] = 1.0

    bf = ml_dtypes.bfloat16
    in_maps = []
    for c in range(NC):
        b, g = c // 4, c % 4
        xT = np.ascontiguousarray(
            x[b].T.reshape(NDT, 128, S).transpose(1, 0, 2)).astype(bf)
        wq = Wqkv[:, g * HD:(g + 1) * HD]
        wk = Wqkv[:, DM + g * HD: DM + (g + 1) * HD]
        wv = Wqkv[:, 2 * DM + g * HD: 2 * DM + (g + 1) * HD]
        w_all = np.ascontiguousarray(
            np.concatenate([wq, wk, wv], axis=1)
            .reshape(NDT, 128, 3 * HD).transpose(1, 0, 2)).astype(bf)
        wo = np.ascontiguousarray(
            Wout[g * HD:(g + 1) * HD, :]
            .reshape(2, 128, DM).transpose(1, 0, 2)).astype(bf)
        bqs = np.ascontiguousarray(
            bqkv[g * HD:(g + 1) * HD].reshape(2, 128).T)         # [128, 2]
        bks = np.ascontiguousarray(
            bqkv[DM + g * HD: DM + (g + 1) * HD].reshape(2, 128).T)
        m = {"xT": xT, "w_all": w_all, "wout": wo, "bq": bqs, "bk": bks,
             "cos_k": cos_k, "sin_k": sin_k, "Pswap": Pm, "onesblk": onesblk,
             "ones2blk": ones2blk, "sel": sel}
        if not shared:
            m["cos_q"] = cos_q
            m["sin_q"] = sin_q
        in_maps.append(m)

    bias_row = (bqkv[2 * DM:] @ Wout + np.asarray(bout, dtype=np.float32)) \
        .astype(np.float32)                                       # [1024]
    return in_maps, shared, float(exp_scale), bias_row


def _install_ntff_shim():
    """Make trace=True usable: this image lacks antenv.axon_hooks; recreate
    it against the baked libaxon_pjrt.so C ABI (no-op if already present)."""
    try:
        from antenv.axon_hooks import get_axon_ntff_profile_hook  # noqa: F401
        return
    except ImportError:
        pass
    try:
        import types, ctypes, contextlib
        import antenv
        lib = ctypes.CDLL("/opt/axon/libaxon_pjrt.so")
        if not hasattr(lib, "axon_start_nrt_profile"):
            raise OSError("no profile symbols")
        lib.axon_start_nrt_profile.argtypes = [ctypes.POINTER(ctypes.c_int64),
                                               ctypes.c_size_t]
        lib.axon_start_nrt_profile.restype = ctypes.c_int64
        lib.axon_stop_nrt_profile.argtypes = [ctypes.c_char_p]
        lib.axon_stop_nrt_profile.restype = ctypes.c_int64

        @contextlib.contextmanager
        def _hook(output_dir, device_ids):
            import jax
            jax.devices()
            if device_ids:
                ids = (ctypes.c_int64 * len(device_ids))(*device_ids)
                rc = lib.axon_start_nrt_profile(ids, len(device_ids))
            else:
                rc = lib.axon_start_nrt_profile(None, 0)
            if rc != 0:
                raise RuntimeError(f"axon_start_nrt_profile rc={rc}")
            try:
                yield
            finally:
                lib.axon_stop_nrt_profile(str(output_dir).encode())

        mod = types.ModuleType("antenv.axon_hooks")
        mod.get_axon_ntff_profile_hook = lambda: _hook
        mod.set_axon_ntff_profile_hook = lambda h: None
        sys.modules["antenv.axon_hooks"] = mod
        antenv.axon_hooks = mod
    except Exception:
        os.environ["BASS_NEVER_TRACE"] = "1"   # degrade: run untraced


def kernel(x, pos, Wqkv, bqkv, Wout, bout, q_scale, k_scale):
    global LAST_RESULTS
    if os.environ.get("BASS_TRACE"):
        _install_ntff_shim()
    in_maps, shared, exp_scale, bias_row = host_prep(
        x, pos, Wqkv, bqkv, Wout, bout, q_scale, k_scale)

    key = (shared, round(exp_scale, 9))
    if key not in _CACHED:
        _CACHED[key] = build_program(exp_scale, shared)
    nc = _CACHED[key]

    res = bass_utils.run_bass_kernel_spmd(
        nc, in_maps, list(range(NC)),
        trace=bool(os.environ.get("BASS_TRACE")))
    LAST_RESULTS = res

    out = np.empty((B, S, DM), dtype=np.float32)
    for b in range(B):
        acc = bias_row[None, :].astype(np.float32).repeat(S, axis=0)
        for g in range(4):
            acc = acc + res.results[b * 4 + g]["outp"].astype(np.float32)
        out[b] = acc
    return out



# revision 51
# speedup vs baseline: 1.1129x; 1.1129x over previous
"""Trainium2 Bass kernel for nn_Attention_32650341384246.

Full attention layer: qkv proj + per-head RMSNorm(q,k) + RoPE + softmax
attention (non-causal) + out proj.  B=2, S=2048, D=1024, H=16, DH=64.

Sharding: 8 cores; core c handles batch c//4, heads [4*(c%4), 4*(c%4)+4)
(data parallel over batch x tensor parallel over heads).  Each core
computes a partial [S, D] output (its heads @ Wout row-slice); the host
sums the 4 partials per batch and adds the (folded) biases.

Device design (per core):
  - x fed pre-transposed+bf16 as xT [128, 8, 2048]  (p + 128*a = model dim)
  - qkv proj emits qT/kT head-major [128 (2 heads x 64), S] directly
    (lhsT = W slice, rhs = xT slice) and v s-major [s, 4*64].
  - RMSNorm in head-major layout: sum(x^2) over d via ones-block matmul
    (f32r), rsqrt = Exp(-0.5*Ln(mean+eps)) on ACT (same table set as the
    softmax Exp -> zero table switches), partition-broadcast via ones
    matmul.
  - RoPE as q_rot = cosT*u + sinT'*swap(u); swap = adjacent-partition
    permutation matmul; cos/sin tables host-built from `pos` with
    q_scale/k_scale folded in; 1/sqrt(dh) folded into the exp scale.
  - scores^T [k, q] bf16 matmuls (K=64, tile_position row groups),
    PSUM [128, 1024] per head, staggered h0/h1 so ACT exp pipelines
    against PE; exp reads PSUM, writes bf16.
  - AV via lhsT = [v | ones] bf16 (M=65): row 64 accumulates sumexp.
  - normalize: gather 4 sumexp rows -> one DVE reciprocal [4, 512],
    select-matrix matmul broadcasts reciprocal rows across partitions.
  - out proj: lhsT = v_mixT bf16, rhs = Wout row-slice bf16.
Heavy matmuls are bf16 (fp32 PSUM accumulate); small helper matmuls
(sumsq / broadcasts / swap) stay float32r.
"""
import sys, os

sys.path.insert(0, "/opt/trn_rl_repo")

import numpy as np
from contextlib import ExitStack

import ml_dtypes
import concourse.bass as bass
import concourse.mybir as mybir
import concourse.tile as tile
from concourse import bacc
from concourse import bass_utils

F32 = mybir.dt.float32
F32R = mybir.dt.float32r
BF16 = mybir.dt.bfloat16
I16 = mybir.dt.int16
AF = mybir.ActivationFunctionType
ALU = mybir.AluOpType

# Schraudolph exp in bf16 bits: bf16(y) ~= exp(u) when int16(y) = A*u + B.
# A = 2^7/ln2; B centered to split the sawtooth error symmetrically.
SCHRAUD_A = 184.6650279
SCHRAUD_B = 16251.0

B, S, DM, H, DH = 2, 2048, 1024, 16, 64
NC = 8
HPC = H // 4          # 4 heads per core
HD = HPC * DH         # 256
NDT = DM // 128       # 8 model-dim tiles
THETA, EPS = 10000.0, 1e-6

LAST_RESULTS = None   # BassKernelResults of the most recent device run
_CACHED = {}


def build_program(exp_scale: float, shared_tables: bool):
    # Schraudolph offload is only range-proven for the rms-normed shared-scale
    # path (|s/sqrt(dh)| <= 8 keeps the int16 bits in [14.7k, 17.8k]).
    use_dve_exp = shared_tables
    nc = bacc.Bacc("TRN2", target_bir_lowering=False, debug=False)

    xT_d = nc.dram_tensor("xT", [128, NDT, S], BF16, kind="ExternalInput")
    w_d = nc.dram_tensor("w_all", [128, NDT, 3 * HD], BF16, kind="ExternalInput")
    wout_d = nc.dram_tensor("wout", [128, 2, DM], BF16, kind="ExternalInput")
    bq_d = nc.dram_tensor("bq", [128, 2], F32, kind="ExternalInput")
    bk_d = nc.dram_tensor("bk", [128, 2], F32, kind="ExternalInput")
    cosk_d = nc.dram_tensor("cos_k", [128, S], F32, kind="ExternalInput")
    sink_d = nc.dram_tensor("sin_k", [128, S], F32, kind="ExternalInput")
    if not shared_tables:
        cosq_d = nc.dram_tensor("cos_q", [128, S], F32, kind="ExternalInput")
        sinq_d = nc.dram_tensor("sin_q", [128, S], F32, kind="ExternalInput")
    P_d = nc.dram_tensor("Pswap", [128, 128], BF16, kind="ExternalInput")
    ob_d = nc.dram_tensor("onesblk", [128, 128], BF16, kind="ExternalInput")
    o2_d = nc.dram_tensor("ones2blk", [128, 128], BF16, kind="ExternalInput")
    sel_d = nc.dram_tensor("sel", [128, 2, 128], F32R, kind="ExternalInput")
    out_d = nc.dram_tensor("outp", [S, DM], BF16, kind="ExternalOutput")

    with tile.TileContext(nc) as tc, ExitStack() as ctx, \
            nc.allow_low_precision(reason="fp32r/bf16 matmul inputs"):
        singles = ctx.enter_context(tc.tile_pool(name="singles", bufs=1))
        tmp = ctx.enter_context(tc.tile_pool(name="tmp", bufs=2))
        expp = ctx.enter_context(tc.tile_pool(name="expp", bufs=4))
        outp = ctx.enter_context(tc.tile_pool(name="outp", bufs=4))

        # --- first-needed loads up front, finest-grained tiles so compute can
        # start as soon as the first s-chunk of x and the q/k weights land ---
        w_qk = [singles.tile([128, 2 * HD], BF16, name=f"wqk{dt}") for dt in range(NDT)]
        w_v = [singles.tile([128, HD], BF16, name=f"wv{dt}") for dt in range(NDT)]
        x_dt = [singles.tile([128, S], BF16, name=f"x{dt}") for dt in range(NDT)]
        for dt in range(NDT):
            nc.sync.dma_start(out=w_qk[dt], in_=w_d.ap()[:, dt, 0:2 * HD])
            nc.sync.dma_start(out=x_dt[dt][:, 0:1024],
                              in_=xT_d.ap()[:, dt, 0:1024])
        for dt in range(NDT):
            nc.sync.dma_start(out=x_dt[dt][:, 1024:2048],
                              in_=xT_d.ap()[:, dt, 1024:2048])
        for dt in range(NDT):
            nc.sync.dma_start(out=w_v[dt], in_=w_d.ap()[:, dt, 2 * HD:3 * HD])

        wout = singles.tile([128, 2, DM], BF16)
        nc.sync.dma_start(out=wout, in_=wout_d.ap())
        bq = singles.tile([128, 2], F32)
        nc.sync.dma_start(out=bq, in_=bq_d.ap())
        bk = singles.tile([128, 2], F32)
        nc.sync.dma_start(out=bk, in_=bk_d.ap())
        cos_k = singles.tile([128, S], F32)
        nc.sync.dma_start(out=cos_k, in_=cosk_d.ap())
        sin_k = singles.tile([128, S], F32)
        nc.sync.dma_start(out=sin_k, in_=sink_d.ap())
        if shared_tables:
            cos_q, sin_q = cos_k, sin_k
        else:
            cos_q = singles.tile([128, S], F32)
            nc.sync.dma_start(out=cos_q, in_=cosq_d.ap())
            sin_q = singles.tile([128, S], F32)
            nc.sync.dma_start(out=sin_q, in_=sinq_d.ap())
        Pm = singles.tile([128, 128], BF16)
        nc.sync.dma_start(out=Pm, in_=P_d.ap())
        onesblk = singles.tile([128, 128], BF16)
        nc.sync.dma_start(out=onesblk, in_=ob_d.ap())
        ones2blk = singles.tile([128, 128], BF16)
        nc.sync.dma_start(out=ones2blk, in_=o2_d.ap())
        sel = singles.tile([128, 2, 128], F32R)
        nc.sync.dma_start(out=sel, in_=sel_d.ap())
        eps_t = singles.tile([128, 1], F32)
        nc.vector.memset(eps_t, EPS)

        qt = [[singles.tile([128, 512], BF16, name=f"qt{t}_{sc}")
               for sc in range(4)] for t in range(2)]
        # k tiles zero-padded per head so scores run as full K=128 matmuls
        # (the other head's partitions hit zero weights) -> phase 2 never
        # switches PE tiling mode.
        ktz = [[[singles.tile([128, 512], BF16, name=f"ktz{t}_{sc}_{h}")
                 for h in range(2)] for sc in range(4)] for t in range(2)]
        for t in range(2):
            for sc in range(4):
                nc.gpsimd.memset(ktz[t][sc][0][64:128, :], 0.0)
                nc.gpsimd.memset(ktz[t][sc][1][0:64, :], 0.0)
        vhat = [singles.tile([128, 4, HPC, 65], BF16, name=f"vhat{sc}")
                for sc in range(4)]
        for sc in range(4):
            nc.vector.memset(vhat[sc][:, :, :, 64:65], 1.0)
        vmix = [[singles.tile([128, 1024], BF16, name=f"vmix{t}_{qh}")
                 for qh in range(2)] for t in range(2)]
        # rs (rsqrt) buffers: zeroed once; Exp writes rows 0-1, rows 2-127
        # stay 0 so the padded K=128 broadcast matmul reads finite zeros.
        rs_bufs = [singles.tile([128, 512], BF16, name=f"rs{i}")
                   for i in range(8)]
        for r in rs_bufs:
            nc.gpsimd.memset(r, 0.0)

        # ---------------- phase 1: qkv + rmsnorm + rope ----------------
        # rope factored as dest = (tt*cos + swap(tt)*sin) * rsqrt_broadcast:
        # the heavy rope work (swap matmul + 3 DVE ops) depends only on tt and
        # pipelines chunk-by-chunk inside stage A; only the final multiply
        # waits for the batched rsqrt.  Per group of 2 sections: stage A
        # (qkv, bias, square, sumsq, rope_raw; Ln lagged 2 chunks so the ACT
        # queue never head-blocks), stage B (8 batched Exps -> one Ln/Exp
        # table swap per group), stage C (rsqrt broadcast + final multiply).
        # Emission g0A g0B g1A g0C g1B v g1C keeps PE dense.  All matmuls are
        # full 128x128 mode (operands zero-padded) -> no PE mode switches.
        sections = (
            ("k", 0, bk, cos_k, sin_k),
            ("q", 0, bq, cos_q, sin_q),
            ("k", 1, bk, cos_k, sin_k),
            ("q", 1, bq, cos_q, sin_q))
        rraws, lnss = {}, {}
        ln_insts = {0: [], 1: []}
        exp_insts = {0: [], 1: []}
        with tc.tile_pool(name="ps1", bufs=1, space="PSUM") as ps1:
            def emit_ln(which, t, sc, gi):
                lns = tmp.tile([2, 512], BF16, tag="lns", bufs=8,
                               name=f"lns{which}{t}_{sc}")
                li = nc.scalar.activation(lns[:, :],
                                          lnss.pop((which, t, sc))[0:2, :],
                                          AF.Ln, bias=eps_t[0:2, :],
                                          scale=1.0 / DH)
                ln_insts[gi].append(li)
                lnss[(which, t, sc, "ln")] = lns

            def stage_a(group, gi):
                todo = []
                for which, t, bias, cosT, sinT in group:
                    off = 0 if which == "q" else HD
                    for sc in range(4):
                        if len(todo) >= 2:   # lag Ln 2 chunks: its pss is long
                            emit_ln(*todo.pop(0), gi)   # done -> no head-block
                        s0 = sc * 512
                        pq = ps1.tile([128, 512], F32, tag="acc", bufs=4,
                                      name=f"pq{which}{t}_{sc}")
                        for dt in range(NDT):
                            nc.tensor.matmul(
                                pq[:, :],
                                w_qk[dt][:, off + t * 128: off + (t + 1) * 128],
                                x_dt[dt][:, s0:s0 + 512],
                                start=(dt == 0), stop=(dt == NDT - 1))
                        tt = tmp.tile([128, 512], BF16, tag="tt", bufs=3,
                                      name=f"tt{which}{t}_{sc}")
                        nc.scalar.activation(tt[:, :], pq[:, :], AF.Identity,
                                             bias=bias[:, t:t + 1], scale=1.0)
                        sq = tmp.tile([128, 512], BF16, tag="sq", bufs=2,
                                      name=f"sq{which}{t}_{sc}")
                        nc.vector.tensor_mul(sq[:, :], tt[:, :], tt[:, :])
                        pss = ps1.tile([128, 512], F32, tag="acc", bufs=4,
                                       name=f"pss{which}{t}_{sc}")
                        nc.tensor.matmul(pss[:, :], onesblk[:, :], sq[:, :],
                                         start=True, stop=True)
                        lnss[(which, t, sc)] = pss
                        psw = ps1.tile([128, 512], F32, tag="work", bufs=4,
                                       name=f"psw{which}{t}_{sc}")
                        nc.tensor.matmul(psw[:, :], Pm[:, :], tt[:, :],
                                         start=True, stop=True)
                        t1 = tmp.tile([128, 512], F32, tag="t1", bufs=2,
                                      name=f"t1{which}{t}_{sc}")
                        nc.vector.tensor_mul(t1[:, :], tt[:, :],
                                             cosT[:, s0:s0 + 512])
                        t2 = tmp.tile([128, 512], F32, tag="t2", bufs=2,
                                      name=f"t2{which}{t}_{sc}")
                        nc.vector.tensor_mul(t2[:, :], psw[:, :],
                                             sinT[:, s0:s0 + 512])
                        rr = tmp.tile([128, 512], BF16, tag="rr", bufs=16,
                                      name=f"rr{which}{t}_{sc}")
                        nc.vector.tensor_add(rr[:, :], t1[:, :], t2[:, :])
                        rraws[(which, t, sc)] = rr
                        todo.append((which, t, sc))
                for item in todo:
                    emit_ln(*item, gi)

            def stage_b(group, gi):
                for j, (which, t, _, _, _) in enumerate(group):
                    for sc in range(4):
                        rs = rs_bufs[j * 4 + sc]
                        ei = nc.scalar.activation(
                            rs[0:2, :], lnss.pop((which, t, sc, "ln"))[:, :],
                            AF.Exp, scale=-0.5)
                        exp_insts[gi].append(ei)

            def stage_c(group):
                for j, (which, t, _, _, _) in enumerate(group):
                    for sc in range(4):
                        rs = rs_bufs[j * 4 + sc]
                        pb = ps1.tile([128, 512], F32, tag="work", bufs=4,
                                      name=f"pb{which}{t}_{sc}")
                        nc.tensor.matmul(pb[:, :], ones2blk[:, :], rs[:, :],
                                         start=True, stop=True)
                        rr = rraws.pop((which, t, sc))
                        if which == "k":
                            nc.vector.tensor_mul(ktz[t][sc][0][0:64, :],
                                                 rr[0:64, :], pb[0:64, :])
                            nc.vector.tensor_mul(ktz[t][sc][1][64:128, :],
                                                 rr[64:128, :], pb[64:128, :])
                        else:
                            nc.vector.tensor_mul(qt[t][sc][:, :],
                                                 rr[:, :], pb[:, :])

            def v_section():
                for sc in range(4):
                    for st in range(4):
                        pv = ps1.tile([128, HD], F32, tag="work", bufs=4,
                                      name=f"pv{sc}_{st}")
                        for dt in range(NDT):
                            nc.tensor.matmul(
                                pv[:, :],
                                x_dt[dt][:, sc * 512 + st * 128: sc * 512 + (st + 1) * 128],
                                w_v[dt][:, :],
                                start=(dt == 0), stop=(dt == NDT - 1))
                        nc.vector.tensor_copy(
                            vhat[sc][:, st, :, 0:64],
                            pv[:, :].rearrange("p (h d) -> p h d", h=HPC))

            g0, g1 = sections[0:2], sections[2:4]
            stage_a(g0, 0)
            stage_b(g0, 0)
            stage_a(g1, 1)
            stage_c(g0)
            stage_b(g1, 1)
            v_section()
            stage_c(g1)

        # Tile's scheduler may interleave ACT instructions across batches,
        # ping-ponging the Ln/Exp table sets (they do not share a set on this
        # target).  Pin the order: g0 lns -> g0 exps -> g1 lns -> g1 exps.
        for ei in exp_insts[0]:
            tile.add_dep_helper(ei.ins, ln_insts[0][-1].ins, sync=False,
                                reason="g0 exps after g0 lns (ACT tables)")
        for li in ln_insts[1]:
            tile.add_dep_helper(li.ins, exp_insts[0][-1].ins, sync=False,
                                reason="g1 lns after g0 exps (ACT tables)")
        for ei in exp_insts[1]:
            tile.add_dep_helper(ei.ins, ln_insts[1][-1].ins, sync=False,
                                reason="g1 exps after g1 lns (ACT tables)")

        # ---------------- phase 2 + 3: attention, out proj per q-half ----------
        # exp split: h0 -> ACT exact exp, h1 -> DVE Schraudolph bit-trick exp
        # (bf16 bits = int16(A*u + B); safe since u = s/sqrt(dh) in [-8, 8]).
        # gpsimd (Pool) absorbs all PSUM->SBUF gather copies.  Out-proj for
        # q rows [qh*1024, +1024) runs right after both pairs finish that qh,
        # reusing the av PSUM banks, so its tail hides under qh1 attention.
        from concourse.dve_ops import (RECIP_APPROX_FAST_CONSTS,
                                       RECIPROCAL_APPROX_FAST)
        _c = RECIP_APPROX_FAST_CONSTS
        se = singles.tile([128, 512], F32, name="se_t")
        nc.gpsimd.memset(se, 1.0)
        with tc.tile_pool(name="ps2", bufs=1, space="PSUM") as ps2:
            for qh in range(2):
                for pair in range(2):
                    q0 = qh * 1024
                    ps_sc = [ps2.tile([128, 1024], F32, tag=f"sc{h}",
                                      name=f"sc{pair}{qh}{h}") for h in range(2)]
                    ps_av = [[ps2.tile([65, 512], F32, tag=f"av{h}{qc}",
                                       name=f"av{pair}{qh}{h}{qc}")
                              for qc in range(2)] for h in range(2)]
                    for kt in range(16):
                        for qc in range(2):
                            for h in range(2):
                                nc.tensor.matmul(
                                    ps_sc[h][:, qc * 512:(qc + 1) * 512],
                                    ktz[pair][kt // 4][h][:, (kt % 4) * 128:(kt % 4 + 1) * 128],
                                    qt[pair][qh * 2 + qc][:, :],
                                    start=True, stop=True)
                        es = []
                        for h in range(2):
                            e = expp.tile([128, 1024], BF16, tag=f"e{h}",
                                          name=f"e{pair}{qh}{h}_{kt}")
                            if h == 1 and kt % 3 != 2 and use_dve_exp:
                                nc.vector.tensor_scalar(
                                    e[:, :].bitcast(I16), ps_sc[h][:, :],
                                    SCHRAUD_A * exp_scale, SCHRAUD_B,
                                    op0=ALU.mult, op1=ALU.add)
                            else:
                                xi = nc.scalar.activation(e[:, :], ps_sc[h][:, :],
                                                          AF.Exp,
                                                          scale=exp_scale)
                                tile.add_dep_helper(
                                    xi.ins, exp_insts[1][-1].ins, sync=False,
                                    reason="phase2 exps after g1 exps (ACT tables)")
                            es.append(e)
                        for h in range(2):
                            head = 2 * pair + h
                            for qc in range(2):
                                nc.tensor.matmul(
                                    ps_av[h][qc][:, :],
                                    vhat[kt // 4][:, kt % 4, head, :],
                                    es[h][:, qc * 512:(qc + 1) * 512],
                                    start=(kt == 0), stop=(kt == 15),
                                    skip_group_check=True)
                    # normalize: batch the 4 sumexp rows -> one reciprocal
                    # (rows live at 32-aligned partitions; rest stay 1.0
                    # so the reciprocal is finite and sel rows zero them)
                    for h in range(2):
                        for qc in range(2):
                            r0 = 32 * (2 * h + qc)
                            nc.vector.tensor_copy(se[r0:r0 + 1, :],
                                                  ps_av[h][qc][64:65, :])
                    recip4 = tmp.tile([128, 512], F32R, tag="recip4",
                                      name=f"rc{pair}{qh}")
                    nc.vector._custom_dve(RECIPROCAL_APPROX_FAST,
                                          out=recip4[:, :], in0=se[:, :],
                                          s0=_c["s0"], s1=_c["s1"],
                                          imm2=_c["imm2"])
                    for qc in range(2):
                        avs2 = tmp.tile([128, 512], F32, tag="avs2", bufs=2,
                                        name=f"avs{pair}{qh}{qc}")
                        for h in range(2):
                            nc.vector.tensor_copy(avs2[h * 64:(h + 1) * 64, :],
                                                  ps_av[h][qc][0:64, :])
                        pb2 = ps2.tile([128, 512], F32, tag=f"av0{qc}",
                                       name=f"nb{pair}{qh}{qc}")
                        nc.tensor.matmul(pb2[:, :], sel[:, qc, :], recip4[:, :],
                                         start=True, stop=True)
                        nc.vector.tensor_mul(
                            vmix[pair][qh][:, qc * 512:(qc + 1) * 512],
                            avs2[:, :], pb2[:, :])
                # out proj for this q-half; po rotates through the av banks
                for stl in range(8):
                    st = qh * 8 + stl
                    for n in range(2):
                        idx = stl * 2 + n
                        po = ps2.tile([128, 512], F32,
                                      tag=f"av{(idx % 4) // 2}{idx % 2}",
                                      name=f"po{st}_{n}")
                        for t in range(2):
                            nc.tensor.matmul(
                                po[:, :],
                                vmix[t][qh][:, stl * 128:(stl + 1) * 128],
                                wout[:, t, n * 512:(n + 1) * 512],
                                start=(t == 0), stop=(t == 1))
                        o = outp.tile([128, 512], BF16, tag="o", name=f"o{st}_{n}")
                        nc.vector.tensor_copy(o[:, :], po[:, :])
                        nc.sync.dma_start(
                            out=out_d.ap()[st * 128:(st + 1) * 128,
                                           n * 512:(n + 1) * 512],
                            in_=o[:, :])

    nc.compile()
    return nc


def host_prep(x, pos, Wqkv, bqkv, Wout, bout, q_scale, k_scale):
    """Build per-core input maps + shared-table decision."""
    x = np.asarray(x, dtype=np.float32)
    pos = np.asarray(pos, dtype=np.float32).reshape(-1)
    Wqkv = np.asarray(Wqkv, dtype=np.float32)
    bqkv = np.asarray(bqkv, dtype=np.float32)
    Wout = np.asarray(Wout, dtype=np.float32)
    q_scale = np.asarray(q_scale, dtype=np.float32)
    k_scale = np.asarray(k_scale, dtype=np.float32)

    shared = bool(np.array_equal(q_scale, k_scale))
    exp_scale = (1.0 / np.sqrt(DH)) if shared else 1.0

    # rope base tables [128, S]
    i_of_p = (np.arange(128) % 64) // 2            # pair index
    sign = np.where(np.arange(128) % 2 == 0, 1.0, -1.0)
    omega = THETA ** (-np.arange(0, DH, 2, dtype=np.float64) / DH)  # [32]
    ang = pos[None, :].astype(np.float64) * omega[:, None]          # [32, S]
    cosb = np.cos(ang)[i_of_p, :]                  # [128, S]
    sinb = np.sin(ang)[i_of_p, :] * sign[:, None]

    def tables(scale_vec, extra):
        sv = np.tile(scale_vec, 2)                 # [128]
        svx = np.tile(scale_vec[np.arange(64) ^ 1], 2)
        cosT = (cosb * sv[:, None] * extra).astype(np.float32)
        sinT = (sinb * svx[:, None] * extra).astype(np.float32)
        return np.ascontiguousarray(cosT), np.ascontiguousarray(sinT)

    cos_k, sin_k = tables(k_scale, 1.0)
    if not shared:
        cos_q, sin_q = tables(q_scale, 1.0 / np.sqrt(DH))

    Pm = np.zeros((128, 128), dtype=ml_dtypes.bfloat16)
    Pm[np.arange(128), np.arange(128) ^ 1] = 1.0
    # zero-padded to full 128x128 so the helper matmuls stay in 128x128 mode
    onesblk = np.zeros((128, 128), dtype=ml_dtypes.bfloat16)
    onesblk[0:64, 0] = 1.0
    onesblk[64:128, 1] = 1.0
    ones2blk = np.zeros((128, 128), dtype=ml_dtypes.bfloat16)
    ones2blk[0, 0:64] = 1.0
    ones2blk[1, 64:128] = 1.0
    # sel[qc]: [4, 128] selecting reciprocal row (h, qc) for partitions h*64..
    sel = np.zeros((128, 2, 128), dtype=np.float32)
    for qc in range(2):
        for h in range(2):
            sel[32 * (2 * h + qc), qc, h * 64:(h + 1) * 64] = 1.0

    bf = ml_dtypes.bfloat16
    in_maps = []
    for c in range(NC):
        b, g = c // 4, c % 4
        xT = np.ascontiguousarray(
            x[b].T.reshape(NDT, 128, S).transpose(1, 0, 2)).astype(bf)
        wq = Wqkv[:, g * HD:(g + 1) * HD]
        wk = Wqkv[:, DM + g * HD: DM + (g + 1) * HD]
        wv = Wqkv[:, 2 * DM + g * HD: 2 * DM + (g + 1) * HD]
        w_all = np.ascontiguousarray(
            np.concatenate([wq, wk, wv], axis=1)
            .reshape(NDT, 128, 3 * HD).transpose(1, 0, 2)).astype(bf)
        wo = np.ascontiguousarray(
            Wout[g * HD:(g + 1) * HD, :]
            .reshape(2, 128, DM).transpose(1, 0, 2)).astype(bf)
        bqs = np.ascontiguousarray(
            bqkv[g * HD:(g + 1) * HD].reshape(2, 128).T)         # [128, 2]
        bks = np.ascontiguousarray(
            bqkv[DM + g * HD: DM + (g + 1) * HD].reshape(2, 128).T)
        m = {"xT": xT, "w_all": w_all, "wout": wo, "bq": bqs, "bk": bks,
             "cos_k": cos_k, "sin_k": sin_k, "Pswap": Pm, "onesblk": onesblk,
             "ones2blk": ones2blk, "sel": sel}
        if not shared:
            m["cos_q"] = cos_q
            m["sin_q"] = sin_q
        in_maps.append(m)

    bias_row = (bqkv[2 * DM:] @ Wout + np.asarray(bout, dtype=np.float32)) \
        .astype(np.float32)                                       # [1024]
    return in_maps, shared, float(exp_scale), bias_row


def _install_ntff_shim():
    """Make trace=True usable: this image lacks antenv.axon_hooks; recreate
    it against the baked libaxon_pjrt.so C ABI (no-op if already present)."""
    try:
        from antenv.axon_hooks import get_axon_ntff_profile_hook  # noqa: F401
        return
    except ImportError:
        pass
    try:
        import types, ctypes, contextlib
        import antenv
        lib = ctypes.CDLL("/opt/axon/libaxon_pjrt.so")
        if not hasattr(lib, "axon_start_nrt_profile"):
            raise OSError("no profile symbols")
        lib.axon_start_nrt_profile.argtypes = [ctypes.POINTER(ctypes.c_int64),
                                               ctypes.c_size_t]
        lib.axon_start_nrt_profile.restype = ctypes.c_int64
        lib.axon_stop_nrt_profile.argtypes = [ctypes.c_char_p]
        lib.axon_stop_nrt_profile.restype = ctypes.c_int64

        @contextlib.contextmanager
        def _hook(output_dir, device_ids):
            import jax
            jax.devices()
            if device_ids:
                ids = (ctypes.c_int64 * len(device_ids))(*device_ids)
                rc = lib.axon_start_nrt_profile(ids, len(device_ids))
            else:
                rc = lib.axon_start_nrt_profile(None, 0)
            if rc != 0:
                raise RuntimeError(f"axon_start_nrt_profile rc={rc}")
            try:
                yield
            finally:
                lib.axon_stop_nrt_profile(str(output_dir).encode())

        mod = types.ModuleType("antenv.axon_hooks")
        mod.get_axon_ntff_profile_hook = lambda: _hook
        mod.set_axon_ntff_profile_hook = lambda h: None
        sys.modules["antenv.axon_hooks"] = mod
        antenv.axon_hooks = mod
    except Exception:
        os.environ["BASS_NEVER_TRACE"] = "1"   # degrade: run untraced


def kernel(x, pos, Wqkv, bqkv, Wout, bout, q_scale, k_scale):
    global LAST_RESULTS
    if os.environ.get("BASS_TRACE"):
        _install_ntff_shim()
    in_maps, shared, exp_scale, bias_row = host_prep(
        x, pos, Wqkv, bqkv, Wout, bout, q_scale, k_scale)

    key = (shared, round(exp_scale, 9))
    if key not in _CACHED:
        _CACHED[key] = build_program(exp_scale, shared)
    nc = _CACHED[key]

    res = bass_utils.run_bass_kernel_spmd(
        nc, in_maps, list(range(NC)),
        trace=bool(os.environ.get("BASS_TRACE")))
    LAST_RESULTS = res

    out = np.empty((B, S, DM), dtype=np.float32)
    for b in range(B):
        acc = bias_row[None, :].astype(np.float32).repeat(S, axis=0)
        for g in range(4):
            acc = acc + res.results[b * 4 + g]["outp"].astype(np.float32)
        out[b] = acc
    return out



# revision 55
# speedup vs baseline: 1.2113x; 1.0884x over previous
"""Trainium2 Bass kernel for nn_Attention_32650341384246.

Full attention layer: qkv proj + per-head RMSNorm(q,k) + RoPE + softmax
attention (non-causal) + out proj.  B=2, S=2048, D=1024, H=16, DH=64.

Sharding: 8 cores; core c handles batch c//4, heads [4*(c%4), 4*(c%4)+4)
(data parallel over batch x tensor parallel over heads).  Each core
computes a partial [S, D] output (its heads @ Wout row-slice); the host
sums the 4 partials per batch and adds the (folded) biases.

Device design (per core):
  - x fed pre-transposed+bf16 as xT [128, 8, 2048]  (p + 128*a = model dim)
  - qkv proj emits qT/kT head-major [128 (2 heads x 64), S] directly
    (lhsT = W slice, rhs = xT slice) and v s-major [s, 4*64].
  - RMSNorm in head-major layout: sum(x^2) over d via ones-block matmul
    (f32r), rsqrt = Exp(-0.5*Ln(mean+eps)) on ACT (same table set as the
    softmax Exp -> zero table switches), partition-broadcast via ones
    matmul.
  - RoPE as q_rot = cosT*u + sinT'*swap(u); swap = adjacent-partition
    permutation matmul; cos/sin tables host-built from `pos` with
    q_scale/k_scale folded in; 1/sqrt(dh) folded into the exp scale.
  - scores^T [k, q] bf16 matmuls (K=64, tile_position row groups),
    PSUM [128, 1024] per head, staggered h0/h1 so ACT exp pipelines
    against PE; exp reads PSUM, writes bf16.
  - AV via lhsT = [v | ones] bf16 (M=65): row 64 accumulates sumexp.
  - normalize: gather 4 sumexp rows -> one DVE reciprocal [4, 512],
    select-matrix matmul broadcasts reciprocal rows across partitions.
  - out proj: lhsT = v_mixT bf16, rhs = Wout row-slice bf16.
Heavy matmuls are bf16 (fp32 PSUM accumulate); small helper matmuls
(sumsq / broadcasts / swap) stay float32r.
"""
import sys, os

sys.path.insert(0, "/opt/trn_rl_repo")

import numpy as np
from contextlib import ExitStack

import ml_dtypes
import concourse.bass as bass
import concourse.mybir as mybir
import concourse.tile as tile
from concourse import bacc
from concourse import bass_utils

F32 = mybir.dt.float32
F32R = mybir.dt.float32r
BF16 = mybir.dt.bfloat16
I16 = mybir.dt.int16
AF = mybir.ActivationFunctionType
ALU = mybir.AluOpType

# Schraudolph exp in bf16 bits: bf16(y) ~= exp(u) when int16(y) = A*u + B.
# A = 2^7/ln2; B centered to split the sawtooth error symmetrically.
SCHRAUD_A = 184.6650279
SCHRAUD_B = 16251.0

B, S, DM, H, DH = 2, 2048, 1024, 16, 64
NC = 8
HPC = H // 4          # 4 heads per core
HD = HPC * DH         # 256
NDT = DM // 128       # 8 model-dim tiles
THETA, EPS = 10000.0, 1e-6

LAST_RESULTS = None   # BassKernelResults of the most recent device run
_CACHED = {}


def build_program(exp_scale: float, shared_tables: bool):
    # Schraudolph offload is only range-proven for the rms-normed shared-scale
    # path (|s/sqrt(dh)| <= 8 keeps the int16 bits in [14.7k, 17.8k]).
    use_dve_exp = shared_tables
    nc = bacc.Bacc("TRN2", target_bir_lowering=False, debug=False)

    xT_d = nc.dram_tensor("xT", [128, NDT, S], BF16, kind="ExternalInput")
    w_d = nc.dram_tensor("w_all", [128, NDT, 3 * HD], BF16, kind="ExternalInput")
    wout_d = nc.dram_tensor("wout", [128, 2, DM], BF16, kind="ExternalInput")
    bq_d = nc.dram_tensor("bq", [128, 2], F32, kind="ExternalInput")
    bk_d = nc.dram_tensor("bk", [128, 2], F32, kind="ExternalInput")
    cosk_d = nc.dram_tensor("cos_k", [128, S], F32, kind="ExternalInput")
    sink_d = nc.dram_tensor("sin_k", [128, S], F32, kind="ExternalInput")
    if not shared_tables:
        cosq_d = nc.dram_tensor("cos_q", [128, S], F32, kind="ExternalInput")
        sinq_d = nc.dram_tensor("sin_q", [128, S], F32, kind="ExternalInput")
    P_d = nc.dram_tensor("Pswap", [128, 128], BF16, kind="ExternalInput")
    ob_d = nc.dram_tensor("onesblk", [128, 128], BF16, kind="ExternalInput")
    o2_d = nc.dram_tensor("ones2blk", [128, 128], BF16, kind="ExternalInput")
    sel_d = nc.dram_tensor("sel", [128, 2, 128], F32R, kind="ExternalInput")
    out_d = nc.dram_tensor("outp", [S, DM], BF16, kind="ExternalOutput")

    with tile.TileContext(nc) as tc, ExitStack() as ctx, \
            nc.allow_low_precision(reason="fp32r/bf16 matmul inputs"):
        singles = ctx.enter_context(tc.tile_pool(name="singles", bufs=1))
        tmp = ctx.enter_context(tc.tile_pool(name="tmp", bufs=2))
        expp = ctx.enter_context(tc.tile_pool(name="expp", bufs=4))
        outp = ctx.enter_context(tc.tile_pool(name="outp", bufs=4))

        # --- first-needed loads up front, finest-grained tiles so compute can
        # start as soon as the first s-chunk of x and the q/k weights land ---
        w_qk = [singles.tile([128, 2 * HD], BF16, name=f"wqk{dt}") for dt in range(NDT)]
        w_v = [singles.tile([128, HD], BF16, name=f"wv{dt}") for dt in range(NDT)]
        x_dt = [singles.tile([128, S], BF16, name=f"x{dt}") for dt in range(NDT)]
        for dt in range(NDT):
            nc.sync.dma_start(out=w_qk[dt], in_=w_d.ap()[:, dt, 0:2 * HD])
            nc.sync.dma_start(out=x_dt[dt][:, 0:1024],
                              in_=xT_d.ap()[:, dt, 0:1024])
        for dt in range(NDT):
            nc.sync.dma_start(out=x_dt[dt][:, 1024:2048],
                              in_=xT_d.ap()[:, dt, 1024:2048])
        for dt in range(NDT):
            nc.sync.dma_start(out=w_v[dt], in_=w_d.ap()[:, dt, 2 * HD:3 * HD])

        wout = singles.tile([128, 2, DM], BF16)
        nc.sync.dma_start(out=wout, in_=wout_d.ap())
        bq = singles.tile([128, 2], F32)
        nc.sync.dma_start(out=bq, in_=bq_d.ap())
        bk = singles.tile([128, 2], F32)
        nc.sync.dma_start(out=bk, in_=bk_d.ap())
        cos_k = singles.tile([128, S], F32)
        nc.sync.dma_start(out=cos_k, in_=cosk_d.ap())
        sin_k = singles.tile([128, S], F32)
        nc.sync.dma_start(out=sin_k, in_=sink_d.ap())
        if shared_tables:
            cos_q, sin_q = cos_k, sin_k
        else:
            cos_q = singles.tile([128, S], F32)
            nc.sync.dma_start(out=cos_q, in_=cosq_d.ap())
            sin_q = singles.tile([128, S], F32)
            nc.sync.dma_start(out=sin_q, in_=sinq_d.ap())
        Pm = singles.tile([128, 128], BF16)
        nc.sync.dma_start(out=Pm, in_=P_d.ap())
        onesblk = singles.tile([128, 128], BF16)
        nc.sync.dma_start(out=onesblk, in_=ob_d.ap())
        ones2blk = singles.tile([128, 128], BF16)
        nc.sync.dma_start(out=ones2blk, in_=o2_d.ap())
        sel = singles.tile([128, 2, 128], F32R)
        nc.sync.dma_start(out=sel, in_=sel_d.ap())
        eps_t = singles.tile([128, 1], F32)
        nc.vector.memset(eps_t, EPS)

        qt = [[singles.tile([128, 512], BF16, name=f"qt{t}_{sc}")
               for sc in range(4)] for t in range(2)]
        # k tiles zero-padded per head so scores run as full K=128 matmuls
        # (the other head's partitions hit zero weights) -> phase 2 never
        # switches PE tiling mode.
        ktz = [[[singles.tile([128, 512], BF16, name=f"ktz{t}_{sc}_{h}")
                 for h in range(2)] for sc in range(4)] for t in range(2)]
        for t in range(2):
            for sc in range(4):
                nc.gpsimd.memset(ktz[t][sc][0][64:128, :], 0.0)
                nc.gpsimd.memset(ktz[t][sc][1][0:64, :], 0.0)
        vhat = [singles.tile([128, 4, HPC, 65], BF16, name=f"vhat{sc}")
                for sc in range(4)]
        for sc in range(4):
            nc.vector.memset(vhat[sc][:, :, :, 64:65], 1.0)
        vmix = [[singles.tile([128, 1024], BF16, name=f"vmix{t}_{qh}")
                 for qh in range(2)] for t in range(2)]
        # rs (rsqrt) buffers: zeroed once; Exp writes rows 0-1, rows 2-127
        # stay 0 so the padded K=128 broadcast matmul reads finite zeros.
        rs_bufs = [singles.tile([128, 512], BF16, name=f"rs{i}")
                   for i in range(8)]
        for r in rs_bufs:
            nc.gpsimd.memset(r, 0.0)

        # ---------------- phase 1: qkv + rmsnorm + rope ----------------
        # rope factored as dest = (tt*cos + swap(tt)*sin) * rsqrt_broadcast:
        # the heavy rope work (swap matmul + 3 DVE ops) depends only on tt and
        # pipelines chunk-by-chunk inside stage A; only the final multiply
        # waits for the batched rsqrt.  Per group of 2 sections: stage A
        # (qkv, bias, square, sumsq, rope_raw; Ln lagged 2 chunks so the ACT
        # queue never head-blocks), stage B (8 batched Exps -> one Ln/Exp
        # table swap per group), stage C (rsqrt broadcast + final multiply).
        # Emission g0A g0B g1A g0C g1B v g1C keeps PE dense.  All matmuls are
        # full 128x128 mode (operands zero-padded) -> no PE mode switches.
        sections = (
            ("k", 0, bk, cos_k, sin_k),
            ("q", 0, bq, cos_q, sin_q),
            ("k", 1, bk, cos_k, sin_k),
            ("q", 1, bq, cos_q, sin_q))
        rraws, lnss = {}, {}
        ln_insts = {0: [], 1: []}
        exp_insts = {0: [], 1: []}
        id_insts = {0: [], 1: []}
        with tc.tile_pool(name="ps1", bufs=1, space="PSUM") as ps1:
            def emit_ln(which, t, sc, gi):
                lns = tmp.tile([2, 512], BF16, tag="lns", bufs=8,
                               name=f"lns{which}{t}_{sc}")
                li = nc.scalar.activation(lns[:, :],
                                          lnss.pop((which, t, sc))[0:2, :],
                                          AF.Ln, bias=eps_t[0:2, :],
                                          scale=1.0 / DH)
                ln_insts[gi].append(li)
                lnss[(which, t, sc, "ln")] = lns

            def stage_a(group, gi):
                todo = []
                for which, t, bias, cosT, sinT in group:
                    off = 0 if which == "q" else HD
                    for sc in range(4):
                        if len(todo) >= 2:   # lag Ln 2 chunks: its pss is long
                            emit_ln(*todo.pop(0), gi)   # done -> no head-block
                        s0 = sc * 512
                        pq = ps1.tile([128, 512], F32, tag="acc", bufs=4,
                                      name=f"pq{which}{t}_{sc}")
                        for dt in range(NDT):
                            nc.tensor.matmul(
                                pq[:, :],
                                w_qk[dt][:, off + t * 128: off + (t + 1) * 128],
                                x_dt[dt][:, s0:s0 + 512],
                                start=(dt == 0), stop=(dt == NDT - 1))
                        tt = tmp.tile([128, 512], BF16, tag="tt", bufs=3,
                                      name=f"tt{which}{t}_{sc}")
                        ii = nc.scalar.activation(tt[:, :], pq[:, :],
                                                  AF.Identity,
                                                  bias=bias[:, t:t + 1],
                                                  scale=1.0)
                        id_insts[gi].append(ii)
                        sq = tmp.tile([128, 512], BF16, tag="sq", bufs=2,
                                      name=f"sq{which}{t}_{sc}")
                        nc.vector.tensor_mul(sq[:, :], tt[:, :], tt[:, :])
                        pss = ps1.tile([128, 512], F32, tag="acc", bufs=4,
                                       name=f"pss{which}{t}_{sc}")
                        nc.tensor.matmul(pss[:, :], onesblk[:, :], sq[:, :],
                                         start=True, stop=True)
                        lnss[(which, t, sc)] = pss
                        psw = ps1.tile([128, 512], F32, tag="work", bufs=4,
                                       name=f"psw{which}{t}_{sc}")
                        nc.tensor.matmul(psw[:, :], Pm[:, :], tt[:, :],
                                         start=True, stop=True)
                        t1 = tmp.tile([128, 512], F32, tag="t1", bufs=2,
                                      name=f"t1{which}{t}_{sc}")
                        nc.vector.tensor_mul(t1[:, :], tt[:, :],
                                             cosT[:, s0:s0 + 512])
                        t2 = tmp.tile([128, 512], F32, tag="t2", bufs=2,
                                      name=f"t2{which}{t}_{sc}")
                        nc.vector.tensor_mul(t2[:, :], psw[:, :],
                                             sinT[:, s0:s0 + 512])
                        rr = tmp.tile([128, 512], BF16, tag="rr", bufs=16,
                                      name=f"rr{which}{t}_{sc}")
                        nc.vector.tensor_add(rr[:, :], t1[:, :], t2[:, :])
                        rraws[(which, t, sc)] = rr
                        todo.append((which, t, sc))
                for item in todo:
                    emit_ln(*item, gi)

            def stage_b(group, gi):
                for j, (which, t, _, _, _) in enumerate(group):
                    for sc in range(4):
                        rs = rs_bufs[j * 4 + sc]
                        ei = nc.scalar.activation(
                            rs[0:2, :], lnss.pop((which, t, sc, "ln"))[:, :],
                            AF.Exp, scale=-0.5)
                        exp_insts[gi].append(ei)

            def stage_c(group):
                for j, (which, t, _, _, _) in enumerate(group):
                    for sc in range(4):
                        rs = rs_bufs[j * 4 + sc]
                        pb = ps1.tile([128, 512], F32, tag="work", bufs=4,
                                      name=f"pb{which}{t}_{sc}")
                        nc.tensor.matmul(pb[:, :], ones2blk[:, :], rs[:, :],
                                         start=True, stop=True)
                        rr = rraws.pop((which, t, sc))
                        if which == "k":
                            nc.vector.tensor_mul(ktz[t][sc][0][0:64, :],
                                                 rr[0:64, :], pb[0:64, :])
                            nc.vector.tensor_mul(ktz[t][sc][1][64:128, :],
                                                 rr[64:128, :], pb[64:128, :])
                        else:
                            nc.vector.tensor_mul(qt[t][sc][:, :],
                                                 rr[:, :], pb[:, :])

            def v_section():
                for sc in range(4):
                    for st in range(4):
                        pv = ps1.tile([128, HD], F32, tag="work", bufs=4,
                                      name=f"pv{sc}_{st}")
                        for dt in range(NDT):
                            nc.tensor.matmul(
                                pv[:, :],
                                x_dt[dt][:, sc * 512 + st * 128: sc * 512 + (st + 1) * 128],
                                w_v[dt][:, :],
                                start=(dt == 0), stop=(dt == NDT - 1))
                        nc.vector.tensor_copy(
                            vhat[sc][:, st, :, 0:64],
                            pv[:, :].rearrange("p (h d) -> p h d", h=HPC))

            g0, g1 = sections[0:2], sections[2:4]
            stage_a(g0, 0)
            stage_b(g0, 0)
            stage_a(g1, 1)
            stage_c(g0)
            stage_b(g1, 1)
            v_section()
            stage_c(g1)

        # Keep the lagged Lns interleaved with the Ids (the scheduler would
        # otherwise sort all Ids first, starving the pss PSUM ring).
        for gi in range(2):
            for j in range(2, 8):
                tile.add_dep_helper(id_insts[gi][j].ins,
                                    ln_insts[gi][j - 2].ins, sync=False,
                                    reason="interleave Id/Ln on ACT queue")
        # Tile's scheduler may interleave ACT instructions across batches,
        # ping-ponging the Ln/Exp table sets (they do not share a set on this
        # target).  Pin the order: g0 lns -> g0 exps -> g1 lns -> g1 exps.
        for ei in exp_insts[0]:
            tile.add_dep_helper(ei.ins, ln_insts[0][-1].ins, sync=False,
                                reason="g0 exps after g0 lns (ACT tables)")
        for li in ln_insts[1]:
            tile.add_dep_helper(li.ins, exp_insts[0][-1].ins, sync=False,
                                reason="g1 lns after g0 exps (ACT tables)")
        for ei in exp_insts[1]:
            tile.add_dep_helper(ei.ins, ln_insts[1][-1].ins, sync=False,
                                reason="g1 exps after g1 lns (ACT tables)")

        # ---------------- phase 2 + 3: attention, out proj per q-half ----------
        # exp split: h0 -> ACT exact exp, h1 -> DVE Schraudolph bit-trick exp
        # (bf16 bits = int16(A*u + B); safe since u = s/sqrt(dh) in [-8, 8]).
        # gpsimd (Pool) absorbs all PSUM->SBUF gather copies.  Out-proj for
        # q rows [qh*1024, +1024) runs right after both pairs finish that qh,
        # reusing the av PSUM banks, so its tail hides under qh1 attention.
        from concourse.dve_ops import (RECIP_APPROX_FAST_CONSTS,
                                       RECIPROCAL_APPROX_FAST)
        _c = RECIP_APPROX_FAST_CONSTS
        se = singles.tile([128, 512], F32, name="se_t")
        nc.gpsimd.memset(se, 1.0)
        with tc.tile_pool(name="ps2", bufs=1, space="PSUM") as ps2:
            for qh in range(2):
                for pair in range(2):
                    q0 = qh * 1024
                    ps_sc = [ps2.tile([128, 1024], F32, tag=f"sc{h}",
                                      name=f"sc{pair}{qh}{h}") for h in range(2)]
                    ps_av = [[ps2.tile([65, 512], F32, tag=f"av{h}{qc}",
                                       name=f"av{pair}{qh}{h}{qc}")
                              for qc in range(2)] for h in range(2)]
                    def emit_av(kt, es):
                        for h in range(2):
                            head = 2 * pair + h
                            for qc in range(2):
                                nc.tensor.matmul(
                                    ps_av[h][qc][:, :],
                                    vhat[kt // 4][:, kt % 4, head, :],
                                    es[h][:, qc * 512:(qc + 1) * 512],
                                    start=(kt == 0), stop=(kt == 15),
                                    skip_group_check=True)

                    # software-pipeline: AV(kt-1) issues after scores(kt), so
                    # the PE never waits on the exp latency
                    prev = None
                    for kt in range(16):
                        for qc in range(2):
                            for h in range(2):
                                nc.tensor.matmul(
                                    ps_sc[h][:, qc * 512:(qc + 1) * 512],
                                    ktz[pair][kt // 4][h][:, (kt % 4) * 128:(kt % 4 + 1) * 128],
                                    qt[pair][qh * 2 + qc][:, :],
                                    start=True, stop=True)
                        es = []
                        for h in range(2):
                            e = expp.tile([128, 1024], BF16, tag=f"e{h}",
                                          name=f"e{pair}{qh}{h}_{kt}")
                            if h == 1 and kt % 3 != 2 and use_dve_exp:
                                nc.vector.tensor_scalar(
                                    e[:, :].bitcast(I16), ps_sc[h][:, :],
                                    SCHRAUD_A * exp_scale, SCHRAUD_B,
                                    op0=ALU.mult, op1=ALU.add)
                            else:
                                xi = nc.scalar.activation(e[:, :], ps_sc[h][:, :],
                                                          AF.Exp,
                                                          scale=exp_scale)
                                tile.add_dep_helper(
                                    xi.ins, exp_insts[1][-1].ins, sync=False,
                                    reason="phase2 exps after g1 exps (ACT tables)")
                            es.append(e)
                        if prev is not None:
                            emit_av(kt - 1, prev)
                        prev = es
                    emit_av(15, prev)
                    # normalize: batch the 4 sumexp rows -> one reciprocal
                    # (rows live at 32-aligned partitions; rest stay 1.0
                    # so the reciprocal is finite and sel rows zero them)
                    for h in range(2):
                        for qc in range(2):
                            r0 = 32 * (2 * h + qc)
                            nc.vector.tensor_copy(se[r0:r0 + 1, :],
                                                  ps_av[h][qc][64:65, :])
                    recip4 = tmp.tile([128, 512], F32R, tag="recip4",
                                      name=f"rc{pair}{qh}")
                    nc.vector._custom_dve(RECIPROCAL_APPROX_FAST,
                                          out=recip4[:, :], in0=se[:, :],
                                          s0=_c["s0"], s1=_c["s1"],
                                          imm2=_c["imm2"])
                    for qc in range(2):
                        avs2 = tmp.tile([128, 512], F32, tag="avs2", bufs=2,
                                        name=f"avs{pair}{qh}{qc}")
                        for h in range(2):
                            nc.vector.tensor_copy(avs2[h * 64:(h + 1) * 64, :],
                                                  ps_av[h][qc][0:64, :])
                        pb2 = ps2.tile([128, 512], F32, tag=f"av0{qc}",
                                       name=f"nb{pair}{qh}{qc}")
                        nc.tensor.matmul(pb2[:, :], sel[:, qc, :], recip4[:, :],
                                         start=True, stop=True)
                        nc.vector.tensor_mul(
                            vmix[pair][qh][:, qc * 512:(qc + 1) * 512],
                            avs2[:, :], pb2[:, :])
                # out proj for this q-half; po rotates through the av banks
                for stl in range(8):
                    st = qh * 8 + stl
                    for n in range(2):
                        idx = stl * 2 + n
                        po = ps2.tile([128, 512], F32,
                                      tag=f"av{(idx % 4) // 2}{idx % 2}",
                                      name=f"po{st}_{n}")
                        for t in range(2):
                            nc.tensor.matmul(
                                po[:, :],
                                vmix[t][qh][:, stl * 128:(stl + 1) * 128],
                                wout[:, t, n * 512:(n + 1) * 512],
                                start=(t == 0), stop=(t == 1))
                        o = outp.tile([128, 512], BF16, tag="o", name=f"o{st}_{n}")
                        nc.vector.tensor_copy(o[:, :], po[:, :])
                        nc.sync.dma_start(
                            out=out_d.ap()[st * 128:(st + 1) * 128,
                                           n * 512:(n + 1) * 512],
                            in_=o[:, :])

    nc.compile()
    return nc


def host_prep(x, pos, Wqkv, bqkv, Wout, bout, q_scale, k_scale):
    """Build per-core input maps + shared-table decision."""
    x = np.asarray(x, dtype=np.float32)
    pos = np.asarray(pos, dtype=np.float32).reshape(-1)
    Wqkv = np.asarray(Wqkv, dtype=np.float32)
    bqkv = np.asarray(bqkv, dtype=np.float32)
    Wout = np.asarray(Wout, dtype=np.float32)
    q_scale = np.asarray(q_scale, dtype=np.float32)
    k_scale = np.asarray(k_scale, dtype=np.float32)

    shared = bool(np.array_equal(q_scale, k_scale))
    exp_scale = (1.0 / np.sqrt(DH)) if shared else 1.0

    # rope base tables [128, S]
    i_of_p = (np.arange(128) % 64) // 2            # pair index
    sign = np.where(np.arange(128) % 2 == 0, 1.0, -1.0)
    omega = THETA ** (-np.arange(0, DH, 2, dtype=np.float64) / DH)  # [32]
    ang = pos[None, :].astype(np.float64) * omega[:, None]          # [32, S]
    cosb = np.cos(ang)[i_of_p, :]                  # [128, S]
    sinb = np.sin(ang)[i_of_p, :] * sign[:, None]

    def tables(scale_vec, extra):
        sv = np.tile(scale_vec, 2)                 # [128]
        svx = np.tile(scale_vec[np.arange(64) ^ 1], 2)
        cosT = (cosb * sv[:, None] * extra).astype(np.float32)
        sinT = (sinb * svx[:, None] * extra).astype(np.float32)
        return np.ascontiguousarray(cosT), np.ascontiguousarray(sinT)

    cos_k, sin_k = tables(k_scale, 1.0)
    if not shared:
        cos_q, sin_q = tables(q_scale, 1.0 / np.sqrt(DH))

    Pm = np.zeros((128, 128), dtype=ml_dtypes.bfloat16)
    Pm[np.arange(128), np.arange(128) ^ 1] = 1.0
    # zero-padded to full 128x128 so the helper matmuls stay in 128x128 mode
    onesblk = np.zeros((128, 128), dtype=ml_dtypes.bfloat16)
    onesblk[0:64, 0] = 1.0
    onesblk[64:128, 1] = 1.0
    ones2blk = np.zeros((128, 128), dtype=ml_dtypes.bfloat16)
    ones2blk[0, 0:64] = 1.0
    ones2blk[1, 64:128] = 1.0
    # sel[qc]: [4, 128] selecting reciprocal row (h, qc) for partitions h*64..
    sel = np.zeros((128, 2, 128), dtype=np.float32)
    for qc in range(2):
        for h in range(2):
            sel[32 * (2 * h + qc), qc, h * 64:(h + 1) * 64] = 1.0

    bf = ml_dtypes.bfloat16
    in_maps = []
    for c in range(NC):
        b, g = c // 4, c % 4
        xT = np.ascontiguousarray(
            x[b].T.reshape(NDT, 128, S).transpose(1, 0, 2)).astype(bf)
        wq = Wqkv[:, g * HD:(g + 1) * HD]
        wk = Wqkv[:, DM + g * HD: DM + (g + 1) * HD]
        wv = Wqkv[:, 2 * DM + g * HD: 2 * DM + (g + 1) * HD]
        w_all = np.ascontiguousarray(
            np.concatenate([wq, wk, wv], axis=1)
            .reshape(NDT, 128, 3 * HD).transpose(1, 0, 2)).astype(bf)
        wo = np.ascontiguousarray(
            Wout[g * HD:(g + 1) * HD, :]
            .reshape(2, 128, DM).transpose(1, 0, 2)).astype(bf)
        bqs = np.ascontiguousarray(
            bqkv[g * HD:(g + 1) * HD].reshape(2, 128).T)         # [128, 2]
        bks = np.ascontiguousarray(
            bqkv[DM + g * HD: DM + (g + 1) * HD].reshape(2, 128).T)
        m = {"xT": xT, "w_all": w_all, "wout": wo, "bq": bqs, "bk": bks,
             "cos_k": cos_k, "sin_k": sin_k, "Pswap": Pm, "onesblk": onesblk,
             "ones2blk": ones2blk, "sel": sel}
        if not shared:
            m["cos_q"] = cos_q
            m["sin_q"] = sin_q
        in_maps.append(m)

    bias_row = (bqkv[2 * DM:] @ Wout + np.asarray(bout, dtype=np.float32)) \
        .astype(np.float32)                                       # [1024]
    return in_maps, shared, float(exp_scale), bias_row


def _install_ntff_shim():
    """Make trace=True usable: this image lacks antenv.axon_hooks; recreate
    it against the baked libaxon_pjrt.so C ABI (no-op if already present)."""
    try:
        from antenv.axon_hooks import get_axon_ntff_profile_hook  # noqa: F401
        return
    except ImportError:
        pass
    try:
        import types, ctypes, contextlib
        import antenv
        lib = ctypes.CDLL("/opt/axon/libaxon_pjrt.so")
        if not hasattr(lib, "axon_start_nrt_profile"):
            raise OSError("no profile symbols")
        lib.axon_start_nrt_profile.argtypes = [ctypes.POINTER(ctypes.c_int64),
                                               ctypes.c_size_t]
        lib.axon_start_nrt_profile.restype = ctypes.c_int64
        lib.axon_stop_nrt_profile.argtypes = [ctypes.c_char_p]
        lib.axon_stop_nrt_profile.restype = ctypes.c_int64

        @contextlib.contextmanager
        def _hook(output_dir, device_ids):
            import jax
            jax.devices()
            if device_ids:
                ids = (ctypes.c_int64 * len(device_ids))(*device_ids)
                rc = lib.axon_start_nrt_profile(ids, len(device_ids))
            else:
                rc = lib.axon_start_nrt_profile(None, 0)
            if rc != 0:
                raise RuntimeError(f"axon_start_nrt_profile rc={rc}")
            try:
                yield
            finally:
                lib.axon_stop_nrt_profile(str(output_dir).encode())

        mod = types.ModuleType("antenv.axon_hooks")
        mod.get_axon_ntff_profile_hook = lambda: _hook
        mod.set_axon_ntff_profile_hook = lambda h: None
        sys.modules["antenv.axon_hooks"] = mod
        antenv.axon_hooks = mod
    except Exception:
        os.environ["BASS_NEVER_TRACE"] = "1"   # degrade: run untraced


def kernel(x, pos, Wqkv, bqkv, Wout, bout, q_scale, k_scale):
    global LAST_RESULTS
    if os.environ.get("BASS_TRACE"):
        _install_ntff_shim()
    in_maps, shared, exp_scale, bias_row = host_prep(
        x, pos, Wqkv, bqkv, Wout, bout, q_scale, k_scale)

    key = (shared, round(exp_scale, 9))
    if key not in _CACHED:
        _CACHED[key] = build_program(exp_scale, shared)
    nc = _CACHED[key]

    res = bass_utils.run_bass_kernel_spmd(
        nc, in_maps, list(range(NC)),
        trace=bool(os.environ.get("BASS_TRACE")))
    LAST_RESULTS = res

    out = np.empty((B, S, DM), dtype=np.float32)
    for b in range(B):
        acc = bias_row[None, :].astype(np.float32).repeat(S, axis=0)
        for g in range(4):
            acc = acc + res.results[b * 4 + g]["outp"].astype(np.float32)
        out[b] = acc
    return out



# revision 57
# speedup vs baseline: 1.2313x; 1.0166x over previous
"""Trainium2 Bass kernel for nn_Attention_32650341384246.

Full attention layer: qkv proj + per-head RMSNorm(q,k) + RoPE + softmax
attention (non-causal) + out proj.  B=2, S=2048, D=1024, H=16, DH=64.

Sharding: 8 cores; core c handles batch c//4, heads [4*(c%4), 4*(c%4)+4)
(data parallel over batch x tensor parallel over heads).  Each core
computes a partial [S, D] output (its heads @ Wout row-slice); the host
sums the 4 partials per batch and adds the (folded) biases.

Device design (per core):
  - x fed pre-transposed+bf16 as xT [128, 8, 2048]  (p + 128*a = model dim)
  - qkv proj emits qT/kT head-major [128 (2 heads x 64), S] directly
    (lhsT = W slice, rhs = xT slice) and v s-major [s, 4*64].
  - RMSNorm in head-major layout: sum(x^2) over d via ones-block matmul
    (f32r), rsqrt = Exp(-0.5*Ln(mean+eps)) on ACT (same table set as the
    softmax Exp -> zero table switches), partition-broadcast via ones
    matmul.
  - RoPE as q_rot = cosT*u + sinT'*swap(u); swap = adjacent-partition
    permutation matmul; cos/sin tables host-built from `pos` with
    q_scale/k_scale folded in; 1/sqrt(dh) folded into the exp scale.
  - scores^T [k, q] bf16 matmuls (K=64, tile_position row groups),
    PSUM [128, 1024] per head, staggered h0/h1 so ACT exp pipelines
    against PE; exp reads PSUM, writes bf16.
  - AV via lhsT = [v | ones] bf16 (M=65): row 64 accumulates sumexp.
  - normalize: gather 4 sumexp rows -> one DVE reciprocal [4, 512],
    select-matrix matmul broadcasts reciprocal rows across partitions.
  - out proj: lhsT = v_mixT bf16, rhs = Wout row-slice bf16.
Heavy matmuls are bf16 (fp32 PSUM accumulate); small helper matmuls
(sumsq / broadcasts / swap) stay float32r.
"""
import sys, os

sys.path.insert(0, "/opt/trn_rl_repo")

import numpy as np
from contextlib import ExitStack

import ml_dtypes
import concourse.bass as bass
import concourse.mybir as mybir
import concourse.tile as tile
from concourse import bacc
from concourse import bass_utils

F32 = mybir.dt.float32
F32R = mybir.dt.float32r
BF16 = mybir.dt.bfloat16
I16 = mybir.dt.int16
AF = mybir.ActivationFunctionType
ALU = mybir.AluOpType

# Schraudolph exp in bf16 bits: bf16(y) ~= exp(u) when int16(y) = A*u + B.
# A = 2^7/ln2; B centered to split the sawtooth error symmetrically.
SCHRAUD_A = 184.6650279
SCHRAUD_B = 16251.0

B, S, DM, H, DH = 2, 2048, 1024, 16, 64
NC = 8
HPC = H // 4          # 4 heads per core
HD = HPC * DH         # 256
NDT = DM // 128       # 8 model-dim tiles
THETA, EPS = 10000.0, 1e-6

LAST_RESULTS = None   # BassKernelResults of the most recent device run
_CACHED = {}


def build_program(exp_scale: float, shared_tables: bool):
    # Schraudolph offload is only range-proven for the rms-normed shared-scale
    # path (|s/sqrt(dh)| <= 8 keeps the int16 bits in [14.7k, 17.8k]).
    use_dve_exp = shared_tables
    nc = bacc.Bacc("TRN2", target_bir_lowering=False, debug=False)

    xT_d = nc.dram_tensor("xT", [128, NDT, S], BF16, kind="ExternalInput")
    w_d = nc.dram_tensor("w_all", [128, NDT, 3 * HD], BF16, kind="ExternalInput")
    wout_d = nc.dram_tensor("wout", [128, 2, DM], BF16, kind="ExternalInput")
    bq_d = nc.dram_tensor("bq", [128, 2], F32, kind="ExternalInput")
    bk_d = nc.dram_tensor("bk", [128, 2], F32, kind="ExternalInput")
    cosk_d = nc.dram_tensor("cos_k", [128, S], F32, kind="ExternalInput")
    sink_d = nc.dram_tensor("sin_k", [128, S], F32, kind="ExternalInput")
    if not shared_tables:
        cosq_d = nc.dram_tensor("cos_q", [128, S], F32, kind="ExternalInput")
        sinq_d = nc.dram_tensor("sin_q", [128, S], F32, kind="ExternalInput")
    P_d = nc.dram_tensor("Pswap", [128, 128], BF16, kind="ExternalInput")
    ob_d = nc.dram_tensor("onesblk", [128, 128], BF16, kind="ExternalInput")
    o2_d = nc.dram_tensor("ones2blk", [128, 128], BF16, kind="ExternalInput")
    sel_d = nc.dram_tensor("sel", [128, 2, 128], F32R, kind="ExternalInput")
    out_d = nc.dram_tensor("outp", [S, DM], BF16, kind="ExternalOutput")

    with tile.TileContext(nc) as tc, ExitStack() as ctx, \
            nc.allow_low_precision(reason="fp32r/bf16 matmul inputs"):
        singles = ctx.enter_context(tc.tile_pool(name="singles", bufs=1))
        tmp = ctx.enter_context(tc.tile_pool(name="tmp", bufs=2))
        expp = ctx.enter_context(tc.tile_pool(name="expp", bufs=4))
        outp = ctx.enter_context(tc.tile_pool(name="outp", bufs=4))

        # --- first-needed loads up front, finest-grained tiles so compute can
        # start as soon as the first s-chunk of x and the q/k weights land ---
        w_qk = [singles.tile([128, 2 * HD], BF16, name=f"wqk{dt}") for dt in range(NDT)]
        w_v = [singles.tile([128, HD], BF16, name=f"wv{dt}") for dt in range(NDT)]
        x_dt = [singles.tile([128, S], BF16, name=f"x{dt}") for dt in range(NDT)]
        for dt in range(NDT):
            nc.sync.dma_start(out=w_qk[dt], in_=w_d.ap()[:, dt, 0:2 * HD])
            nc.sync.dma_start(out=x_dt[dt][:, 0:1024],
                              in_=xT_d.ap()[:, dt, 0:1024])
        for dt in range(NDT):
            nc.sync.dma_start(out=x_dt[dt][:, 1024:2048],
                              in_=xT_d.ap()[:, dt, 1024:2048])
        for dt in range(NDT):
            nc.sync.dma_start(out=w_v[dt], in_=w_d.ap()[:, dt, 2 * HD:3 * HD])

        wout = singles.tile([128, 2, DM], BF16)
        nc.sync.dma_start(out=wout, in_=wout_d.ap())
        bq = singles.tile([128, 2], F32)
        nc.sync.dma_start(out=bq, in_=bq_d.ap())
        bk = singles.tile([128, 2], F32)
        nc.sync.dma_start(out=bk, in_=bk_d.ap())
        cos_k = singles.tile([128, S], F32)
        nc.sync.dma_start(out=cos_k, in_=cosk_d.ap())
        sin_k = singles.tile([128, S], F32)
        nc.sync.dma_start(out=sin_k, in_=sink_d.ap())
        if shared_tables:
            cos_q, sin_q = cos_k, sin_k
        else:
            cos_q = singles.tile([128, S], F32)
            nc.sync.dma_start(out=cos_q, in_=cosq_d.ap())
            sin_q = singles.tile([128, S], F32)
            nc.sync.dma_start(out=sin_q, in_=sinq_d.ap())
        Pm = singles.tile([128, 128], BF16)
        nc.sync.dma_start(out=Pm, in_=P_d.ap())
        onesblk = singles.tile([128, 128], BF16)
        nc.sync.dma_start(out=onesblk, in_=ob_d.ap())
        ones2blk = singles.tile([128, 128], BF16)
        nc.sync.dma_start(out=ones2blk, in_=o2_d.ap())
        sel = singles.tile([128, 2, 128], F32R)
        nc.sync.dma_start(out=sel, in_=sel_d.ap())
        eps_t = singles.tile([128, 1], F32)
        nc.vector.memset(eps_t, EPS)

        qt = [[singles.tile([128, 512], BF16, name=f"qt{t}_{sc}")
               for sc in range(4)] for t in range(2)]
        # k tiles zero-padded per head so scores run as full K=128 matmuls
        # (the other head's partitions hit zero weights) -> phase 2 never
        # switches PE tiling mode.
        ktz = [[[singles.tile([128, 512], BF16, name=f"ktz{t}_{sc}_{h}")
                 for h in range(2)] for sc in range(4)] for t in range(2)]
        for t in range(2):
            for sc in range(4):
                nc.gpsimd.memset(ktz[t][sc][0][64:128, :], 0.0)
                nc.gpsimd.memset(ktz[t][sc][1][0:64, :], 0.0)
        vhat = [singles.tile([128, 4, HPC, 65], BF16, name=f"vhat{sc}")
                for sc in range(4)]
        for sc in range(4):
            nc.vector.memset(vhat[sc][:, :, :, 64:65], 1.0)
        vmix = [[singles.tile([128, 1024], BF16, name=f"vmix{t}_{qh}")
                 for qh in range(2)] for t in range(2)]
        # rs (rsqrt) buffers: zeroed once; Exp writes rows 0-1, rows 2-127
        # stay 0 so the padded K=128 broadcast matmul reads finite zeros.
        rs_bufs = [singles.tile([128, 512], BF16, name=f"rs{i}")
                   for i in range(8)]
        for r in rs_bufs:
            nc.gpsimd.memset(r, 0.0)

        # ---------------- phase 1: qkv + rmsnorm + rope ----------------
        # rope factored as dest = (tt*cos + swap(tt)*sin) * rsqrt_broadcast:
        # the heavy rope work (swap matmul + 3 DVE ops) depends only on tt and
        # pipelines chunk-by-chunk inside stage A; only the final multiply
        # waits for the batched rsqrt.  Per group of 2 sections: stage A
        # (qkv, bias, square, sumsq, rope_raw; Ln lagged 2 chunks so the ACT
        # queue never head-blocks), stage B (8 batched Exps -> one Ln/Exp
        # table swap per group), stage C (rsqrt broadcast + final multiply).
        # Emission g0A g0B g1A g0C g1B v g1C keeps PE dense.  All matmuls are
        # full 128x128 mode (operands zero-padded) -> no PE mode switches.
        sections = (
            ("k", 0, bk, cos_k, sin_k),
            ("q", 0, bq, cos_q, sin_q),
            ("k", 1, bk, cos_k, sin_k),
            ("q", 1, bq, cos_q, sin_q))
        rraws, lnss = {}, {}
        ln_insts = {0: [], 1: []}
        exp_insts = {0: [], 1: []}
        id_insts = {0: [], 1: []}
        with tc.tile_pool(name="ps1", bufs=1, space="PSUM") as ps1:
            def emit_ln(which, t, sc, gi):
                lns = tmp.tile([2, 512], BF16, tag="lns", bufs=8,
                               name=f"lns{which}{t}_{sc}")
                li = nc.scalar.activation(lns[:, :],
                                          lnss.pop((which, t, sc))[0:2, :],
                                          AF.Ln, bias=eps_t[0:2, :],
                                          scale=1.0 / DH)
                ln_insts[gi].append(li)
                lnss[(which, t, sc, "ln")] = lns

            def stage_a(group, gi):
                todo = []
                for sc in range(4):       # sc-outer: matches x DMA arrival
                    for which, t, bias, cosT, sinT in group:
                        off = 0 if which == "q" else HD
                        if len(todo) >= 2:   # lag Ln 2 chunks: its pss is long
                            emit_ln(*todo.pop(0), gi)   # done -> no head-block
                        s0 = sc * 512
                        pq = ps1.tile([128, 512], F32, tag="acc", bufs=4,
                                      name=f"pq{which}{t}_{sc}")
                        for dt in range(NDT):
                            nc.tensor.matmul(
                                pq[:, :],
                                w_qk[dt][:, off + t * 128: off + (t + 1) * 128],
                                x_dt[dt][:, s0:s0 + 512],
                                start=(dt == 0), stop=(dt == NDT - 1))
                        tt = tmp.tile([128, 512], BF16, tag="tt", bufs=3,
                                      name=f"tt{which}{t}_{sc}")
                        ii = nc.scalar.activation(tt[:, :], pq[:, :],
                                                  AF.Identity,
                                                  bias=bias[:, t:t + 1],
                                                  scale=1.0)
                        id_insts[gi].append(ii)
                        sq = tmp.tile([128, 512], BF16, tag="sq", bufs=2,
                                      name=f"sq{which}{t}_{sc}")
                        nc.vector.tensor_mul(sq[:, :], tt[:, :], tt[:, :])
                        pss = ps1.tile([128, 512], F32, tag="acc", bufs=4,
                                       name=f"pss{which}{t}_{sc}")
                        nc.tensor.matmul(pss[:, :], onesblk[:, :], sq[:, :],
                                         start=True, stop=True)
                        lnss[(which, t, sc)] = pss
                        psw = ps1.tile([128, 512], F32, tag="work", bufs=4,
                                       name=f"psw{which}{t}_{sc}")
                        nc.tensor.matmul(psw[:, :], Pm[:, :], tt[:, :],
                                         start=True, stop=True)
                        t1 = tmp.tile([128, 512], F32, tag="t1", bufs=2,
                                      name=f"t1{which}{t}_{sc}")
                        nc.vector.tensor_mul(t1[:, :], tt[:, :],
                                             cosT[:, s0:s0 + 512])
                        t2 = tmp.tile([128, 512], F32, tag="t2", bufs=2,
                                      name=f"t2{which}{t}_{sc}")
                        nc.vector.tensor_mul(t2[:, :], psw[:, :],
                                             sinT[:, s0:s0 + 512])
                        rr = tmp.tile([128, 512], BF16, tag="rr", bufs=16,
                                      name=f"rr{which}{t}_{sc}")
                        nc.vector.tensor_add(rr[:, :], t1[:, :], t2[:, :])
                        rraws[(which, t, sc)] = rr
                        todo.append((which, t, sc))
                for item in todo:
                    emit_ln(*item, gi)

            def stage_b(group, gi):
                for j, (which, t, _, _, _) in enumerate(group):
                    for sc in range(4):
                        rs = rs_bufs[j * 4 + sc]
                        ei = nc.scalar.activation(
                            rs[0:2, :], lnss.pop((which, t, sc, "ln"))[:, :],
                            AF.Exp, scale=-0.5)
                        exp_insts[gi].append(ei)

            def stage_c(group):
                for j, (which, t, _, _, _) in enumerate(group):
                    for sc in range(4):
                        rs = rs_bufs[j * 4 + sc]
                        pb = ps1.tile([128, 512], F32, tag="work", bufs=4,
                                      name=f"pb{which}{t}_{sc}")
                        nc.tensor.matmul(pb[:, :], ones2blk[:, :], rs[:, :],
                                         start=True, stop=True)
                        rr = rraws.pop((which, t, sc))
                        if which == "k":
                            nc.vector.tensor_mul(ktz[t][sc][0][0:64, :],
                                                 rr[0:64, :], pb[0:64, :])
                            nc.vector.tensor_mul(ktz[t][sc][1][64:128, :],
                                                 rr[64:128, :], pb[64:128, :])
                        else:
                            nc.vector.tensor_mul(qt[t][sc][:, :],
                                                 rr[:, :], pb[:, :])

            def v_section():
                for sc in range(4):
                    for st in range(4):
                        pv = ps1.tile([128, HD], F32, tag="work", bufs=4,
                                      name=f"pv{sc}_{st}")
                        for dt in range(NDT):
                            nc.tensor.matmul(
                                pv[:, :],
                                x_dt[dt][:, sc * 512 + st * 128: sc * 512 + (st + 1) * 128],
                                w_v[dt][:, :],
                                start=(dt == 0), stop=(dt == NDT - 1))
                        nc.vector.tensor_copy(
                            vhat[sc][:, st, :, 0:64],
                            pv[:, :].rearrange("p (h d) -> p h d", h=HPC))

            g0, g1 = sections[0:2], sections[2:4]
            stage_a(g0, 0)
            stage_b(g0, 0)
            stage_a(g1, 1)
            stage_c(g0)
            stage_b(g1, 1)
            v_section()
            stage_c(g1)

        # Keep the lagged Lns interleaved with the Ids (the scheduler would
        # otherwise sort all Ids first, starving the pss PSUM ring).
        for gi in range(2):
            for j in range(2, 8):
                tile.add_dep_helper(id_insts[gi][j].ins,
                                    ln_insts[gi][j - 2].ins, sync=False,
                                    reason="interleave Id/Ln on ACT queue")
        # Tile's scheduler may interleave ACT instructions across batches,
        # ping-ponging the Ln/Exp table sets (they do not share a set on this
        # target).  Pin the order: g0 lns -> g0 exps -> g1 lns -> g1 exps.
        for ei in exp_insts[0]:
            tile.add_dep_helper(ei.ins, ln_insts[0][-1].ins, sync=False,
                                reason="g0 exps after g0 lns (ACT tables)")
        for li in ln_insts[1]:
            tile.add_dep_helper(li.ins, exp_insts[0][-1].ins, sync=False,
                                reason="g1 lns after g0 exps (ACT tables)")
        for ei in exp_insts[1]:
            tile.add_dep_helper(ei.ins, ln_insts[1][-1].ins, sync=False,
                                reason="g1 exps after g1 lns (ACT tables)")

        # ---------------- phase 2 + 3: attention, out proj per q-half ----------
        # exp split: h0 -> ACT exact exp, h1 -> DVE Schraudolph bit-trick exp
        # (bf16 bits = int16(A*u + B); safe since u = s/sqrt(dh) in [-8, 8]).
        # gpsimd (Pool) absorbs all PSUM->SBUF gather copies.  Out-proj for
        # q rows [qh*1024, +1024) runs right after both pairs finish that qh,
        # reusing the av PSUM banks, so its tail hides under qh1 attention.
        from concourse.dve_ops import (RECIP_APPROX_FAST_CONSTS,
                                       RECIPROCAL_APPROX_FAST)
        _c = RECIP_APPROX_FAST_CONSTS
        se = singles.tile([128, 512], F32, name="se_t")
        nc.gpsimd.memset(se, 1.0)
        with tc.tile_pool(name="ps2", bufs=1, space="PSUM") as ps2:
            for qh in range(2):
                for pair in range(2):
                    q0 = qh * 1024
                    ps_sc = [ps2.tile([128, 1024], F32, tag=f"sc{h}",
                                      name=f"sc{pair}{qh}{h}") for h in range(2)]
                    ps_av = [[ps2.tile([65, 512], F32, tag=f"av{h}{qc}",
                                       name=f"av{pair}{qh}{h}{qc}")
                              for qc in range(2)] for h in range(2)]
                    def emit_av(kt, es):
                        for h in range(2):
                            head = 2 * pair + h
                            for qc in range(2):
                                nc.tensor.matmul(
                                    ps_av[h][qc][:, :],
                                    vhat[kt // 4][:, kt % 4, head, :],
                                    es[h][:, qc * 512:(qc + 1) * 512],
                                    start=(kt == 0), stop=(kt == 15),
                                    skip_group_check=True)

                    # software-pipeline: AV(kt-1) issues after scores(kt), so
                    # the PE never waits on the exp latency
                    prev = None
                    for kt in range(16):
                        for qc in range(2):
                            for h in range(2):
                                nc.tensor.matmul(
                                    ps_sc[h][:, qc * 512:(qc + 1) * 512],
                                    ktz[pair][kt // 4][h][:, (kt % 4) * 128:(kt % 4 + 1) * 128],
                                    qt[pair][qh * 2 + qc][:, :],
                                    start=True, stop=True)
                        es = []
                        for h in range(2):
                            e = expp.tile([128, 1024], BF16, tag=f"e{h}",
                                          name=f"e{pair}{qh}{h}_{kt}")
                            if h == 1 and kt % 3 != 2 and use_dve_exp:
                                nc.vector.tensor_scalar(
                                    e[:, :].bitcast(I16), ps_sc[h][:, :],
                                    SCHRAUD_A * exp_scale, SCHRAUD_B,
                                    op0=ALU.mult, op1=ALU.add)
                            else:
                                xi = nc.scalar.activation(e[:, :], ps_sc[h][:, :],
                                                          AF.Exp,
                                                          scale=exp_scale)
                                tile.add_dep_helper(
                                    xi.ins, exp_insts[1][-1].ins, sync=False,
                                    reason="phase2 exps after g1 exps (ACT tables)")
                            es.append(e)
                        if prev is not None:
                            emit_av(kt - 1, prev)
                        prev = es
                    emit_av(15, prev)
                    # normalize: batch the 4 sumexp rows -> one reciprocal
                    # (rows live at 32-aligned partitions; rest stay 1.0
                    # so the reciprocal is finite and sel rows zero them)
                    for h in range(2):
                        for qc in range(2):
                            r0 = 32 * (2 * h + qc)
                            nc.vector.tensor_copy(se[r0:r0 + 1, :],
                                                  ps_av[h][qc][64:65, :])
                    recip4 = tmp.tile([128, 512], F32R, tag="recip4",
                                      name=f"rc{pair}{qh}")
                    nc.vector._custom_dve(RECIPROCAL_APPROX_FAST,
                                          out=recip4[:, :], in0=se[:, :],
                                          s0=_c["s0"], s1=_c["s1"],
                                          imm2=_c["imm2"])
                    for qc in range(2):
                        avs2 = tmp.tile([128, 512], F32, tag="avs2", bufs=2,
                                        name=f"avs{pair}{qh}{qc}")
                        for h in range(2):
                            nc.vector.tensor_copy(avs2[h * 64:(h + 1) * 64, :],
                                                  ps_av[h][qc][0:64, :])
                        pb2 = ps2.tile([128, 512], F32, tag=f"av0{qc}",
                                       name=f"nb{pair}{qh}{qc}")
                        nc.tensor.matmul(pb2[:, :], sel[:, qc, :], recip4[:, :],
                                         start=True, stop=True)
                        nc.vector.tensor_mul(
                            vmix[pair][qh][:, qc * 512:(qc + 1) * 512],
                            avs2[:, :], pb2[:, :])
                # out proj for this q-half; po rotates through the av banks
                for stl in range(8):
                    st = qh * 8 + stl
                    for n in range(2):
                        idx = stl * 2 + n
                        po = ps2.tile([128, 512], F32,
                                      tag=f"av{(idx % 4) // 2}{idx % 2}",
                                      name=f"po{st}_{n}")
                        for t in range(2):
                            nc.tensor.matmul(
                                po[:, :],
                                vmix[t][qh][:, stl * 128:(stl + 1) * 128],
                                wout[:, t, n * 512:(n + 1) * 512],
                                start=(t == 0), stop=(t == 1))
                        o = outp.tile([128, 512], BF16, tag="o", name=f"o{st}_{n}")
                        nc.scalar.activation(o[:, :], po[:, :], AF.Identity)
                        nc.sync.dma_start(
                            out=out_d.ap()[st * 128:(st + 1) * 128,
                                           n * 512:(n + 1) * 512],
                            in_=o[:, :])

    nc.compile()
    return nc


def host_prep(x, pos, Wqkv, bqkv, Wout, bout, q_scale, k_scale):
    """Build per-core input maps + shared-table decision."""
    x = np.asarray(x, dtype=np.float32)
    pos = np.asarray(pos, dtype=np.float32).reshape(-1)
    Wqkv = np.asarray(Wqkv, dtype=np.float32)
    bqkv = np.asarray(bqkv, dtype=np.float32)
    Wout = np.asarray(Wout, dtype=np.float32)
    q_scale = np.asarray(q_scale, dtype=np.float32)
    k_scale = np.asarray(k_scale, dtype=np.float32)

    shared = bool(np.array_equal(q_scale, k_scale))
    exp_scale = (1.0 / np.sqrt(DH)) if shared else 1.0

    # rope base tables [128, S]
    i_of_p = (np.arange(128) % 64) // 2            # pair index
    sign = np.where(np.arange(128) % 2 == 0, 1.0, -1.0)
    omega = THETA ** (-np.arange(0, DH, 2, dtype=np.float64) / DH)  # [32]
    ang = pos[None, :].astype(np.float64) * omega[:, None]          # [32, S]
    cosb = np.cos(ang)[i_of_p, :]                  # [128, S]
    sinb = np.sin(ang)[i_of_p, :] * sign[:, None]

    def tables(scale_vec, extra):
        sv = np.tile(scale_vec, 2)                 # [128]
        svx = np.tile(scale_vec[np.arange(64) ^ 1], 2)
        cosT = (cosb * sv[:, None] * extra).astype(np.float32)
        sinT = (sinb * svx[:, None] * extra).astype(np.float32)
        return np.ascontiguousarray(cosT), np.ascontiguousarray(sinT)

    cos_k, sin_k = tables(k_scale, 1.0)
    if not shared:
        cos_q, sin_q = tables(q_scale, 1.0 / np.sqrt(DH))

    Pm = np.zeros((128, 128), dtype=ml_dtypes.bfloat16)
    Pm[np.arange(128), np.arange(128) ^ 1] = 1.0
    # zero-padded to full 128x128 so the helper matmuls stay in 128x128 mode
    onesblk = np.zeros((128, 128), dtype=ml_dtypes.bfloat16)
    onesblk[0:64, 0] = 1.0
    onesblk[64:128, 1] = 1.0
    ones2blk = np.zeros((128, 128), dtype=ml_dtypes.bfloat16)
    ones2blk[0, 0:64] = 1.0
    ones2blk[1, 64:128] = 1.0
    # sel[qc]: [4, 128] selecting reciprocal row (h, qc) for partitions h*64..
    sel = np.zeros((128, 2, 128), dtype=np.float32)
    for qc in range(2):
        for h in range(2):
            sel[32 * (2 * h + qc), qc, h * 64:(h + 1) * 64] = 1.0

    bf = ml_dtypes.bfloat16
    in_maps = []
    for c in range(NC):
        b, g = c // 4, c % 4
        xT = np.ascontiguousarray(
            x[b].T.reshape(NDT, 128, S).transpose(1, 0, 2)).astype(bf)
        wq = Wqkv[:, g * HD:(g + 1) * HD]
        wk = Wqkv[:, DM + g * HD: DM + (g + 1) * HD]
        wv = Wqkv[:, 2 * DM + g * HD: 2 * DM + (g + 1) * HD]
        w_all = np.ascontiguousarray(
            np.concatenate([wq, wk, wv], axis=1)
            .reshape(NDT, 128, 3 * HD).transpose(1, 0, 2)).astype(bf)
        wo = np.ascontiguousarray(
            Wout[g * HD:(g + 1) * HD, :]
            .reshape(2, 128, DM).transpose(1, 0, 2)).astype(bf)
        bqs = np.ascontiguousarray(
            bqkv[g * HD:(g + 1) * HD].reshape(2, 128).T)         # [128, 2]
        bks = np.ascontiguousarray(
            bqkv[DM + g * HD: DM + (g + 1) * HD].reshape(2, 128).T)
        m = {"xT": xT, "w_all": w_all, "wout": wo, "bq": bqs, "bk": bks,
             "cos_k": cos_k, "sin_k": sin_k, "Pswap": Pm, "onesblk": onesblk,
             "ones2blk": ones2blk, "sel": sel}
        if not shared:
            m["cos_q"] = cos_q
            m["sin_q"] = sin_q
        in_maps.append(m)

    bias_row = (bqkv[2 * DM:] @ Wout + np.asarray(bout, dtype=np.float32)) \
        .astype(np.float32)                                       # [1024]
    return in_maps, shared, float(exp_scale), bias_row


def _install_ntff_shim():
    """Make trace=True usable: this image lacks antenv.axon_hooks; recreate
    it against the baked libaxon_pjrt.so C ABI (no-op if already present)."""
    try:
        from antenv.axon_hooks import get_axon_ntff_profile_hook  # noqa: F401
        return
    except ImportError:
        pass
    try:
        import types, ctypes, contextlib
        import antenv
        lib = ctypes.CDLL("/opt/axon/libaxon_pjrt.so")
        if not hasattr(lib, "axon_start_nrt_profile"):
            raise OSError("no profile symbols")
        lib.axon_start_nrt_profile.argtypes = [ctypes.POINTER(ctypes.c_int64),
                                               ctypes.c_size_t]
        lib.axon_start_nrt_profile.restype = ctypes.c_int64
        lib.axon_stop_nrt_profile.argtypes = [ctypes.c_char_p]
        lib.axon_stop_nrt_profile.restype = ctypes.c_int64

        @contextlib.contextmanager
        def _hook(output_dir, device_ids):
            import jax
            jax.devices()
            if device_ids:
                ids = (ctypes.c_int64 * len(device_ids))(*device_ids)
                rc = lib.axon_start_nrt_profile(ids, len(device_ids))
            else:
                rc = lib.axon_start_nrt_profile(None, 0)
            if rc != 0:
                raise RuntimeError(f"axon_start_nrt_profile rc={rc}")
            try:
                yield
            finally:
                lib.axon_stop_nrt_profile(str(output_dir).encode())

        mod = types.ModuleType("antenv.axon_hooks")
        mod.get_axon_ntff_profile_hook = lambda: _hook
        mod.set_axon_ntff_profile_hook = lambda h: None
        sys.modules["antenv.axon_hooks"] = mod
        antenv.axon_hooks = mod
    except Exception:
        os.environ["BASS_NEVER_TRACE"] = "1"   # degrade: run untraced


def kernel(x, pos, Wqkv, bqkv, Wout, bout, q_scale, k_scale):
    global LAST_RESULTS
    if os.environ.get("BASS_TRACE"):
        _install_ntff_shim()
    in_maps, shared, exp_scale, bias_row = host_prep(
        x, pos, Wqkv, bqkv, Wout, bout, q_scale, k_scale)

    key = (shared, round(exp_scale, 9))
    if key not in _CACHED:
        _CACHED[key] = build_program(exp_scale, shared)
    nc = _CACHED[key]

    res = bass_utils.run_bass_kernel_spmd(
        nc, in_maps, list(range(NC)),
        trace=bool(os.environ.get("BASS_TRACE")))
    LAST_RESULTS = res

    out = np.empty((B, S, DM), dtype=np.float32)
    for b in range(B):
        acc = bias_row[None, :].astype(np.float32).repeat(S, axis=0)
        for g in range(4):
            acc = acc + res.results[b * 4 + g]["outp"].astype(np.float32)
        out[b] = acc
    return out



# revision 61
# speedup vs baseline: 1.2735x; 1.0342x over previous
"""Trainium2 Bass kernel for nn_Attention_32650341384246.

Full attention layer: qkv proj + per-head RMSNorm(q,k) + RoPE + softmax
attention (non-causal) + out proj.  B=2, S=2048, D=1024, H=16, DH=64.

Sharding: 8 cores; core c handles batch c//4, heads [4*(c%4), 4*(c%4)+4)
(data parallel over batch x tensor parallel over heads).  Each core
computes a partial [S, D] output (its heads @ Wout row-slice); the host
sums the 4 partials per batch and adds the (folded) biases.

Device design (per core):
  - x fed pre-transposed+bf16 as xT [128, 8, 2048]  (p + 128*a = model dim)
  - qkv proj emits qT/kT head-major [128 (2 heads x 64), S] directly
    (lhsT = W slice, rhs = xT slice) and v s-major [s, 4*64].
  - RMSNorm in head-major layout: sum(x^2) over d via ones-block matmul
    (f32r), rsqrt = Exp(-0.5*Ln(mean+eps)) on ACT (same table set as the
    softmax Exp -> zero table switches), partition-broadcast via ones
    matmul.
  - RoPE as q_rot = cosT*u + sinT'*swap(u); swap = adjacent-partition
    permutation matmul; cos/sin tables host-built from `pos` with
    q_scale/k_scale folded in; 1/sqrt(dh) folded into the exp scale.
  - scores^T [k, q] bf16 matmuls (K=64, tile_position row groups),
    PSUM [128, 1024] per head, staggered h0/h1 so ACT exp pipelines
    against PE; exp reads PSUM, writes bf16.
  - AV via lhsT = [v | ones] bf16 (M=65): row 64 accumulates sumexp.
  - normalize: gather 4 sumexp rows -> one DVE reciprocal [4, 512],
    select-matrix matmul broadcasts reciprocal rows across partitions.
  - out proj: lhsT = v_mixT bf16, rhs = Wout row-slice bf16.
Heavy matmuls are bf16 (fp32 PSUM accumulate); small helper matmuls
(sumsq / broadcasts / swap) stay float32r.
"""
import sys, os

sys.path.insert(0, "/opt/trn_rl_repo")

import numpy as np
from contextlib import ExitStack

import ml_dtypes
import concourse.bass as bass
import concourse.mybir as mybir
import concourse.tile as tile
from concourse import bacc
from concourse import bass_utils

F32 = mybir.dt.float32
F32R = mybir.dt.float32r
BF16 = mybir.dt.bfloat16
I16 = mybir.dt.int16
AF = mybir.ActivationFunctionType
ALU = mybir.AluOpType

# Schraudolph exp in bf16 bits: bf16(y) ~= exp(u) when int16(y) = A*u + B.
# A = 2^7/ln2; B centered to split the sawtooth error symmetrically.
SCHRAUD_A = 184.6650279
SCHRAUD_B = 16251.0

B, S, DM, H, DH = 2, 2048, 1024, 16, 64
NC = 8
HPC = H // 4          # 4 heads per core
HD = HPC * DH         # 256
NDT = DM // 128       # 8 model-dim tiles
THETA, EPS = 10000.0, 1e-6

LAST_RESULTS = None   # BassKernelResults of the most recent device run
_CACHED = {}


def build_program(exp_scale: float, shared_tables: bool):
    # Schraudolph offload is only range-proven for the rms-normed shared-scale
    # path (|s/sqrt(dh)| <= 8 keeps the int16 bits in [14.7k, 17.8k]).
    use_dve_exp = shared_tables
    nc = bacc.Bacc("TRN2", target_bir_lowering=False, debug=False)

    xT_d = nc.dram_tensor("xT", [128, NDT, S], BF16, kind="ExternalInput")
    w_d = nc.dram_tensor("w_all", [128, NDT, 3 * HD], BF16, kind="ExternalInput")
    wout_d = nc.dram_tensor("wout", [128, 2, DM], BF16, kind="ExternalInput")
    bq_d = nc.dram_tensor("bq", [128, 2], F32, kind="ExternalInput")
    bk_d = nc.dram_tensor("bk", [128, 2], F32, kind="ExternalInput")
    cosk_d = nc.dram_tensor("cos_k", [128, S], F32, kind="ExternalInput")
    sink_d = nc.dram_tensor("sin_k", [128, S], F32, kind="ExternalInput")
    if not shared_tables:
        cosq_d = nc.dram_tensor("cos_q", [128, S], F32, kind="ExternalInput")
        sinq_d = nc.dram_tensor("sin_q", [128, S], F32, kind="ExternalInput")
    P_d = nc.dram_tensor("Pswap", [128, 128], BF16, kind="ExternalInput")
    ob_d = nc.dram_tensor("onesblk", [128, 128], BF16, kind="ExternalInput")
    o2_d = nc.dram_tensor("ones2blk", [128, 128], BF16, kind="ExternalInput")
    sel_d = nc.dram_tensor("sel", [128, 2, 128], F32R, kind="ExternalInput")
    out_d = nc.dram_tensor("outp", [S, DM], BF16, kind="ExternalOutput")

    with tile.TileContext(nc) as tc, ExitStack() as ctx, \
            nc.allow_low_precision(reason="fp32r/bf16 matmul inputs"):
        singles = ctx.enter_context(tc.tile_pool(name="singles", bufs=1))
        tmp = ctx.enter_context(tc.tile_pool(name="tmp", bufs=2))
        expp = ctx.enter_context(tc.tile_pool(name="expp", bufs=4))
        outp = ctx.enter_context(tc.tile_pool(name="outp", bufs=4))

        # --- first-needed loads up front, finest-grained tiles so compute can
        # start as soon as the first s-chunk of x and the q/k weights land ---
        w_qk = [singles.tile([128, 2 * HD], BF16, name=f"wqk{dt}") for dt in range(NDT)]
        w_v = [singles.tile([128, HD], BF16, name=f"wv{dt}") for dt in range(NDT)]
        x_dt = [singles.tile([128, S], BF16, name=f"x{dt}") for dt in range(NDT)]
        for dt in range(NDT):
            nc.sync.dma_start(out=w_qk[dt], in_=w_d.ap()[:, dt, 0:2 * HD])
            nc.sync.dma_start(out=x_dt[dt][:, 0:1024],
                              in_=xT_d.ap()[:, dt, 0:1024])
        for dt in range(NDT):
            nc.sync.dma_start(out=x_dt[dt][:, 1024:2048],
                              in_=xT_d.ap()[:, dt, 1024:2048])
        for dt in range(NDT):
            nc.sync.dma_start(out=w_v[dt], in_=w_d.ap()[:, dt, 2 * HD:3 * HD])

        wout = singles.tile([128, 2, DM], BF16)
        nc.sync.dma_start(out=wout, in_=wout_d.ap())
        bq = singles.tile([128, 2], F32)
        nc.sync.dma_start(out=bq, in_=bq_d.ap())
        bk = singles.tile([128, 2], F32)
        nc.sync.dma_start(out=bk, in_=bk_d.ap())
        cos_k = singles.tile([128, S], F32)
        nc.sync.dma_start(out=cos_k, in_=cosk_d.ap())
        sin_k = singles.tile([128, S], F32)
        nc.sync.dma_start(out=sin_k, in_=sink_d.ap())
        if shared_tables:
            cos_q, sin_q = cos_k, sin_k
        else:
            cos_q = singles.tile([128, S], F32)
            nc.sync.dma_start(out=cos_q, in_=cosq_d.ap())
            sin_q = singles.tile([128, S], F32)
            nc.sync.dma_start(out=sin_q, in_=sinq_d.ap())
        Pm = singles.tile([128, 128], BF16)
        nc.sync.dma_start(out=Pm, in_=P_d.ap())
        onesblk = singles.tile([128, 128], BF16)
        nc.sync.dma_start(out=onesblk, in_=ob_d.ap())
        ones2blk = singles.tile([128, 128], BF16)
        nc.sync.dma_start(out=ones2blk, in_=o2_d.ap())
        sel = singles.tile([128, 2, 128], F32R)
        nc.sync.dma_start(out=sel, in_=sel_d.ap())
        eps_t = singles.tile([128, 1], F32)
        nc.vector.memset(eps_t, EPS)

        qt = [[singles.tile([128, 512], BF16, name=f"qt{t}_{sc}")
               for sc in range(4)] for t in range(2)]
        # k tiles zero-padded per head so scores run as full K=128 matmuls
        # (the other head's partitions hit zero weights) -> phase 2 never
        # switches PE tiling mode.
        ktz = [[[singles.tile([128, 512], BF16, name=f"ktz{t}_{sc}_{h}")
                 for h in range(2)] for sc in range(4)] for t in range(2)]
        for t in range(2):
            for sc in range(4):
                nc.gpsimd.memset(ktz[t][sc][0][64:128, :], 0.0)
                nc.gpsimd.memset(ktz[t][sc][1][0:64, :], 0.0)
        vhat = [singles.tile([128, 4, HPC, 65], BF16, name=f"vhat{sc}")
                for sc in range(4)]
        for sc in range(4):
            nc.vector.memset(vhat[sc][:, :, :, 64:65], 1.0)
        vmix = [[singles.tile([128, 1024], BF16, name=f"vmix{t}_{qh}")
                 for qh in range(2)] for t in range(2)]
        # rs (rsqrt) buffers: zeroed once; Exp writes rows 0-1, rows 2-127
        # stay 0 so the padded K=128 broadcast matmul reads finite zeros.
        rs_bufs = [singles.tile([128, 512], BF16, name=f"rs{i}")
                   for i in range(16)]
        for r in rs_bufs:
            nc.gpsimd.memset(r, 0.0)
        # dummy Ln: pulls the Ln table-set DMA to t~0, before it would have
        # to queue behind the 25us input-DMA flood
        dummy = singles.tile([1, 1], F32, name="dummy")
        nc.scalar.activation(dummy[:, :], eps_t[0:1, 0:1], AF.Ln)

        # ---------------- phase 1: qkv + rmsnorm + rope ----------------
        # rope factored as dest = (tt*cos + swap(tt)*sin) * rsqrt_broadcast:
        # the heavy rope work (swap matmul + 3 DVE ops) depends only on tt and
        # pipelines chunk-by-chunk inside stage A; only the final multiply
        # waits for the batched rsqrt.  Per group of 2 sections: stage A
        # (qkv, bias, square, sumsq, rope_raw; Ln lagged 2 chunks so the ACT
        # queue never head-blocks), stage B (8 batched Exps -> one Ln/Exp
        # table swap per group), stage C (rsqrt broadcast + final multiply).
        # Emission g0A g0B g1A g0C g1B v g1C keeps PE dense.  All matmuls are
        # full 128x128 mode (operands zero-padded) -> no PE mode switches.
        sections = (
            ("k", 0, bk, cos_k, sin_k),
            ("q", 0, bq, cos_q, sin_q),
            ("k", 1, bk, cos_k, sin_k),
            ("q", 1, bq, cos_q, sin_q))
        rraws, lnss = {}, {}
        ln_insts = {0: [], 1: []}
        exp_insts = {0: [], 1: []}
        id_insts = {0: [], 1: []}
        with tc.tile_pool(name="ps1", bufs=1, space="PSUM") as ps1:
            def emit_ln(which, t, sc, gi):
                lns = tmp.tile([2, 512], BF16, tag="lns", bufs=16,
                               name=f"lns{which}{t}_{sc}")
                li = nc.scalar.activation(lns[:, :],
                                          lnss.pop((which, t, sc))[0:2, :],
                                          AF.Ln, bias=eps_t[0:2, :],
                                          scale=1.0 / DH)
                ln_insts[gi].append(li)
                lnss[(which, t, sc, "ln")] = lns

            def stage_a(group, gi):
                todo = []
                for sc in range(4):       # sc-outer: matches x DMA arrival
                    for which, t, bias, cosT, sinT in group:
                        off = 0 if which == "q" else HD
                        if len(todo) >= 2:   # lag Ln 2 chunks: its pss is long
                            emit_ln(*todo.pop(0), gi)   # done -> no head-block
                        s0 = sc * 512
                        pq = ps1.tile([128, 512], F32, tag="acc", bufs=4,
                                      name=f"pq{which}{t}_{sc}")
                        for dt in range(NDT):
                            nc.tensor.matmul(
                                pq[:, :],
                                w_qk[dt][:, off + t * 128: off + (t + 1) * 128],
                                x_dt[dt][:, s0:s0 + 512],
                                start=(dt == 0), stop=(dt == NDT - 1))
                        tt = tmp.tile([128, 512], BF16, tag="tt", bufs=3,
                                      name=f"tt{which}{t}_{sc}")
                        ii = nc.scalar.activation(tt[:, :], pq[:, :],
                                                  AF.Identity,
                                                  bias=bias[:, t:t + 1],
                                                  scale=1.0)
                        id_insts[gi].append(ii)
                        sq = tmp.tile([128, 512], BF16, tag="sq", bufs=2,
                                      name=f"sq{which}{t}_{sc}")
                        nc.vector.tensor_mul(sq[:, :], tt[:, :], tt[:, :])
                        pss = ps1.tile([128, 512], F32, tag="acc", bufs=4,
                                       name=f"pss{which}{t}_{sc}")
                        nc.tensor.matmul(pss[:, :], onesblk[:, :], sq[:, :],
                                         start=True, stop=True)
                        lnss[(which, t, sc)] = pss
                        psw = ps1.tile([128, 512], F32, tag="work", bufs=4,
                                       name=f"psw{which}{t}_{sc}")
                        nc.tensor.matmul(psw[:, :], Pm[:, :], tt[:, :],
                                         start=True, stop=True)
                        t1 = tmp.tile([128, 512], F32, tag="t1", bufs=2,
                                      name=f"t1{which}{t}_{sc}")
                        nc.vector.tensor_mul(t1[:, :], tt[:, :],
                                             cosT[:, s0:s0 + 512])
                        t2 = tmp.tile([128, 512], F32, tag="t2", bufs=2,
                                      name=f"t2{which}{t}_{sc}")
                        nc.vector.tensor_mul(t2[:, :], psw[:, :],
                                             sinT[:, s0:s0 + 512])
                        rr = tmp.tile([128, 512], BF16, tag="rr", bufs=16,
                                      name=f"rr{which}{t}_{sc}")
                        nc.vector.tensor_add(rr[:, :], t1[:, :], t2[:, :])
                        rraws[(which, t, sc)] = rr
                        todo.append((which, t, sc))
                for item in todo:
                    emit_ln(*item, gi)

            def stage_b(group, gi):
                for j, (which, t, _, _, _) in enumerate(group):
                    for sc in range(4):
                        rs = rs_bufs[j * 4 + sc]
                        ei = nc.scalar.activation(
                            rs[0:2, :], lnss.pop((which, t, sc, "ln"))[:, :],
                            AF.Exp, scale=-0.5)
                        exp_insts[gi].append(ei)

            def stage_c(group):
                for j, (which, t, _, _, _) in enumerate(group):
                    for sc in range(4):
                        rs = rs_bufs[j * 4 + sc]
                        pb = ps1.tile([128, 512], F32, tag="work", bufs=4,
                                      name=f"pb{which}{t}_{sc}")
                        nc.tensor.matmul(pb[:, :], ones2blk[:, :], rs[:, :],
                                         start=True, stop=True)
                        rr = rraws.pop((which, t, sc))
                        if which == "k":
                            nc.vector.tensor_mul(ktz[t][sc][0][0:64, :],
                                                 rr[0:64, :], pb[0:64, :])
                            nc.vector.tensor_mul(ktz[t][sc][1][64:128, :],
                                                 rr[64:128, :], pb[64:128, :])
                        else:
                            nc.vector.tensor_mul(qt[t][sc][:, :],
                                                 rr[:, :], pb[:, :])

            def v_section():
                for sc in range(4):
                    for st in range(4):
                        pv = ps1.tile([128, HD], F32, tag="work", bufs=4,
                                      name=f"pv{sc}_{st}")
                        for dt in range(NDT):
                            nc.tensor.matmul(
                                pv[:, :],
                                x_dt[dt][:, sc * 512 + st * 128: sc * 512 + (st + 1) * 128],
                                w_v[dt][:, :],
                                start=(dt == 0), stop=(dt == NDT - 1))
                        nc.vector.tensor_copy(
                            vhat[sc][:, st, :, 0:64],
                            pv[:, :].rearrange("p (h d) -> p h d", h=HPC))

            stage_a(sections, 0)
            stage_b(sections, 0)
            v_section()
            stage_c(sections)

        # Keep the lagged Lns interleaved with the Ids (the scheduler would
        # otherwise sort all Ids first, starving the pss PSUM ring), and all
        # Exps after all Lns (Ln/Exp sit in different ACT table sets on this
        # target: batching leaves exactly 2 loads for the whole kernel).
        for j in range(2, 16):
            tile.add_dep_helper(id_insts[0][j].ins,
                                ln_insts[0][j - 2].ins, sync=False,
                                reason="interleave Id/Ln on ACT queue")
        for ei in exp_insts[0]:
            tile.add_dep_helper(ei.ins, ln_insts[0][-1].ins, sync=False,
                                reason="exps after lns (ACT tables)")

        # ---------------- phase 2 + 3: attention, out proj per q-half ----------
        # exp split: h0 -> ACT exact exp, h1 -> DVE Schraudolph bit-trick exp
        # (bf16 bits = int16(A*u + B); safe since u = s/sqrt(dh) in [-8, 8]).
        # gpsimd (Pool) absorbs all PSUM->SBUF gather copies.  Out-proj for
        # q rows [qh*1024, +1024) runs right after both pairs finish that qh,
        # reusing the av PSUM banks, so its tail hides under qh1 attention.
        from concourse.dve_ops import (RECIP_APPROX_FAST_CONSTS,
                                       RECIPROCAL_APPROX_FAST)
        _c = RECIP_APPROX_FAST_CONSTS
        se = singles.tile([128, 512], F32, name="se_t")
        nc.gpsimd.memset(se, 1.0)
        with tc.tile_pool(name="ps2", bufs=1, space="PSUM") as ps2:
            for qh in range(2):
                for pair in range(2):
                    q0 = qh * 1024
                    ps_sc = [ps2.tile([128, 1024], F32, tag=f"sc{h}",
                                      name=f"sc{pair}{qh}{h}") for h in range(2)]
                    ps_av = [[ps2.tile([65, 512], F32, tag=f"av{h}{qc}",
                                       name=f"av{pair}{qh}{h}{qc}")
                              for qc in range(2)] for h in range(2)]
                    def emit_av(kt, es):
                        for h in range(2):
                            head = 2 * pair + h
                            for qc in range(2):
                                nc.tensor.matmul(
                                    ps_av[h][qc][:, :],
                                    vhat[kt // 4][:, kt % 4, head, :],
                                    es[h][:, qc * 512:(qc + 1) * 512],
                                    start=(kt == 0), stop=(kt == 15),
                                    skip_group_check=True)

                    # software-pipeline: AV(kt-1) issues after scores(kt), so
                    # the PE never waits on the exp latency
                    prev = None
                    for kt in range(16):
                        for qc in range(2):
                            for h in range(2):
                                nc.tensor.matmul(
                                    ps_sc[h][:, qc * 512:(qc + 1) * 512],
                                    ktz[pair][kt // 4][h][:, (kt % 4) * 128:(kt % 4 + 1) * 128],
                                    qt[pair][qh * 2 + qc][:, :],
                                    start=True, stop=True)
                        es = []
                        for h in range(2):
                            e = expp.tile([128, 1024], BF16, tag=f"e{h}",
                                          name=f"e{pair}{qh}{h}_{kt}")
                            if h == 1 and kt % 3 != 2 and use_dve_exp:
                                nc.vector.tensor_scalar(
                                    e[:, :].bitcast(I16), ps_sc[h][:, :],
                                    SCHRAUD_A * exp_scale, SCHRAUD_B,
                                    op0=ALU.mult, op1=ALU.add)
                            else:
                                xi = nc.scalar.activation(e[:, :], ps_sc[h][:, :],
                                                          AF.Exp,
                                                          scale=exp_scale)
                                tile.add_dep_helper(
                                    xi.ins, exp_insts[0][-1].ins, sync=False,
                                    reason="phase2 exps after rsqrt exps (ACT tables)")
                            es.append(e)
                        if prev is not None:
                            emit_av(kt - 1, prev)
                        prev = es
                    emit_av(15, prev)
                    # normalize: batch the 4 sumexp rows -> one reciprocal
                    # (rows live at 32-aligned partitions; rest stay 1.0
                    # so the reciprocal is finite and sel rows zero them)
                    for h in range(2):
                        for qc in range(2):
                            r0 = 32 * (2 * h + qc)
                            nc.vector.tensor_copy(se[r0:r0 + 1, :],
                                                  ps_av[h][qc][64:65, :])
                    recip4 = tmp.tile([128, 512], F32R, tag="recip4",
                                      name=f"rc{pair}{qh}")
                    nc.vector._custom_dve(RECIPROCAL_APPROX_FAST,
                                          out=recip4[:, :], in0=se[:, :],
                                          s0=_c["s0"], s1=_c["s1"],
                                          imm2=_c["imm2"])
                    for qc in range(2):
                        avs2 = tmp.tile([128, 512], F32, tag="avs2", bufs=2,
                                        name=f"avs{pair}{qh}{qc}")
                        for h in range(2):
                            nc.vector.tensor_copy(avs2[h * 64:(h + 1) * 64, :],
                                                  ps_av[h][qc][0:64, :])
                        pb2 = ps2.tile([128, 512], F32, tag=f"av0{qc}",
                                       name=f"nb{pair}{qh}{qc}")
                        nc.tensor.matmul(pb2[:, :], sel[:, qc, :], recip4[:, :],
                                         start=True, stop=True)
                        nc.vector.tensor_mul(
                            vmix[pair][qh][:, qc * 512:(qc + 1) * 512],
                            avs2[:, :], pb2[:, :])
                # out proj for this q-half; po rotates through the av banks
                for stl in range(8):
                    st = qh * 8 + stl
                    for n in range(2):
                        idx = stl * 2 + n
                        po = ps2.tile([128, 512], F32,
                                      tag=f"av{(idx % 4) // 2}{idx % 2}",
                                      name=f"po{st}_{n}")
                        for t in range(2):
                            nc.tensor.matmul(
                                po[:, :],
                                vmix[t][qh][:, stl * 128:(stl + 1) * 128],
                                wout[:, t, n * 512:(n + 1) * 512],
                                start=(t == 0), stop=(t == 1))
                        o = outp.tile([128, 512], BF16, tag="o", name=f"o{st}_{n}")
                        nc.scalar.activation(o[:, :], po[:, :], AF.Identity)
                        nc.sync.dma_start(
                            out=out_d.ap()[st * 128:(st + 1) * 128,
                                           n * 512:(n + 1) * 512],
                            in_=o[:, :])

    nc.compile()
    return nc


def host_prep(x, pos, Wqkv, bqkv, Wout, bout, q_scale, k_scale):
    """Build per-core input maps + shared-table decision."""
    x = np.asarray(x, dtype=np.float32)
    pos = np.asarray(pos, dtype=np.float32).reshape(-1)
    Wqkv = np.asarray(Wqkv, dtype=np.float32)
    bqkv = np.asarray(bqkv, dtype=np.float32)
    Wout = np.asarray(Wout, dtype=np.float32)
    q_scale = np.asarray(q_scale, dtype=np.float32)
    k_scale = np.asarray(k_scale, dtype=np.float32)

    shared = bool(np.array_equal(q_scale, k_scale))
    exp_scale = (1.0 / np.sqrt(DH)) if shared else 1.0

    # rope base tables [128, S]
    i_of_p = (np.arange(128) % 64) // 2            # pair index
    sign = np.where(np.arange(128) % 2 == 0, 1.0, -1.0)
    omega = THETA ** (-np.arange(0, DH, 2, dtype=np.float64) / DH)  # [32]
    ang = pos[None, :].astype(np.float64) * omega[:, None]          # [32, S]
    cosb = np.cos(ang)[i_of_p, :]                  # [128, S]
    sinb = np.sin(ang)[i_of_p, :] * sign[:, None]

    def tables(scale_vec, extra):
        sv = np.tile(scale_vec, 2)                 # [128]
        svx = np.tile(scale_vec[np.arange(64) ^ 1], 2)
        cosT = (cosb * sv[:, None] * extra).astype(np.float32)
        sinT = (sinb * svx[:, None] * extra).astype(np.float32)
        return np.ascontiguousarray(cosT), np.ascontiguousarray(sinT)

    cos_k, sin_k = tables(k_scale, 1.0)
    if not shared:
        cos_q, sin_q = tables(q_scale, 1.0 / np.sqrt(DH))

    Pm = np.zeros((128, 128), dtype=ml_dtypes.bfloat16)
    Pm[np.arange(128), np.arange(128) ^ 1] = 1.0
    # zero-padded to full 128x128 so the helper matmuls stay in 128x128 mode
    onesblk = np.zeros((128, 128), dtype=ml_dtypes.bfloat16)
    onesblk[0:64, 0] = 1.0
    onesblk[64:128, 1] = 1.0
    ones2blk = np.zeros((128, 128), dtype=ml_dtypes.bfloat16)
    ones2blk[0, 0:64] = 1.0
    ones2blk[1, 64:128] = 1.0
    # sel[qc]: [4, 128] selecting reciprocal row (h, qc) for partitions h*64..
    sel = np.zeros((128, 2, 128), dtype=np.float32)
    for qc in range(2):
        for h in range(2):
            sel[32 * (2 * h + qc), qc, h * 64:(h + 1) * 64] = 1.0

    bf = ml_dtypes.bfloat16
    in_maps = []
    for c in range(NC):
        b, g = c // 4, c % 4
        xT = np.ascontiguousarray(
            x[b].T.reshape(NDT, 128, S).transpose(1, 0, 2)).astype(bf)
        wq = Wqkv[:, g * HD:(g + 1) * HD]
        wk = Wqkv[:, DM + g * HD: DM + (g + 1) * HD]
        wv = Wqkv[:, 2 * DM + g * HD: 2 * DM + (g + 1) * HD]
        w_all = np.ascontiguousarray(
            np.concatenate([wq, wk, wv], axis=1)
            .reshape(NDT, 128, 3 * HD).transpose(1, 0, 2)).astype(bf)
        wo = np.ascontiguousarray(
            Wout[g * HD:(g + 1) * HD, :]
            .reshape(2, 128, DM).transpose(1, 0, 2)).astype(bf)
        bqs = np.ascontiguousarray(
            bqkv[g * HD:(g + 1) * HD].reshape(2, 128).T)         # [128, 2]
        bks = np.ascontiguousarray(
            bqkv[DM + g * HD: DM + (g + 1) * HD].reshape(2, 128).T)
        m = {"xT": xT, "w_all": w_all, "wout": wo, "bq": bqs, "bk": bks,
             "cos_k": cos_k, "sin_k": sin_k, "Pswap": Pm, "onesblk": onesblk,
             "ones2blk": ones2blk, "sel": sel}
        if not shared:
            m["cos_q"] = cos_q
            m["sin_q"] = sin_q
        in_maps.append(m)

    bias_row = (bqkv[2 * DM:] @ Wout + np.asarray(bout, dtype=np.float32)) \
        .astype(np.float32)                                       # [1024]
    return in_maps, shared, float(exp_scale), bias_row


def _install_ntff_shim():
    """Make trace=True usable: this image lacks antenv.axon_hooks; recreate
    it against the baked libaxon_pjrt.so C ABI (no-op if already present)."""
    try:
        from antenv.axon_hooks import get_axon_ntff_profile_hook  # noqa: F401
        return
    except ImportError:
        pass
    try:
        import types, ctypes, contextlib
        import antenv
        lib = ctypes.CDLL("/opt/axon/libaxon_pjrt.so")
        if not hasattr(lib, "axon_start_nrt_profile"):
            raise OSError("no profile symbols")
        lib.axon_start_nrt_profile.argtypes = [ctypes.POINTER(ctypes.c_int64),
                                               ctypes.c_size_t]
        lib.axon_start_nrt_profile.restype = ctypes.c_int64
        lib.axon_stop_nrt_profile.argtypes = [ctypes.c_char_p]
        lib.axon_stop_nrt_profile.restype = ctypes.c_int64

        @contextlib.contextmanager
        def _hook(output_dir, device_ids):
            import jax
            jax.devices()
            if device_ids:
                ids = (ctypes.c_int64 * len(device_ids))(*device_ids)
                rc = lib.axon_start_nrt_profile(ids, len(device_ids))
            else:
                rc = lib.axon_start_nrt_profile(None, 0)
            if rc != 0:
                raise RuntimeError(f"axon_start_nrt_profile rc={rc}")
            try:
                yield
            finally:
                lib.axon_stop_nrt_profile(str(output_dir).encode())

        mod = types.ModuleType("antenv.axon_hooks")
        mod.get_axon_ntff_profile_hook = lambda: _hook
        mod.set_axon_ntff_profile_hook = lambda h: None
        sys.modules["antenv.axon_hooks"] = mod
        antenv.axon_hooks = mod
    except Exception:
        os.environ["BASS_NEVER_TRACE"] = "1"   # degrade: run untraced


def kernel(x, pos, Wqkv, bqkv, Wout, bout, q_scale, k_scale):
    global LAST_RESULTS
    if os.environ.get("BASS_TRACE"):
        _install_ntff_shim()
    in_maps, shared, exp_scale, bias_row = host_prep(
        x, pos, Wqkv, bqkv, Wout, bout, q_scale, k_scale)

    key = (shared, round(exp_scale, 9))
    if key not in _CACHED:
        _CACHED[key] = build_program(exp_scale, shared)
    nc = _CACHED[key]

    res = bass_utils.run_bass_kernel_spmd(
        nc, in_maps, list(range(NC)),
        trace=bool(os.environ.get("BASS_TRACE")))
    LAST_RESULTS = res

    out = np.empty((B, S, DM), dtype=np.float32)
    for b in range(B):
        acc = bias_row[None, :].astype(np.float32).repeat(S, axis=0)
        for g in range(4):
            acc = acc + res.results[b * 4 + g]["outp"].astype(np.float32)
        out[b] = acc
    return out

